# revision 1
# baseline (speedup 1.0000x reference)
"""Trainium2 Bass kernel for CDSQN (3-layer GCN + hypernetwork Q-head).

Contract: kernel(**inputs) takes the FULL unsharded inputs (numpy) and
returns the FULL [B] float32 output. Internally shards across 8
NeuronCores: nodes/edges by dst range (4000 nodes = 4 graphs per core),
GCN weights replicated, hypernetwork Wg1 sharded over the node axis.
"""
import sys

sys.path.insert(0, "/opt/trn_rl_repo")

import numpy as np

import concourse.bacc as bacc
import concourse.bass as bass
import concourse.mybir as mybir
import concourse.tile as tile
from concourse.ap import AP
from concourse.masks import make_identity

# ---- problem constants (hardcoded per spec) ----
B = 32
N_PER = 1000
TOTAL = B * N_PER          # 32000
E = 512000
F = 128                    # node_feat_dim == hidden_dim
D = 64
NH = 3
EPS = 1e-6

NCORES = 8
NODES_PER_CORE = TOTAL // NCORES       # 4000
GRAPHS_PER_CORE = B // NCORES          # 4
TILE_N = 125                           # dst nodes per output tile
TILES_PER_CORE = NODES_PER_CORE // TILE_N   # 32
TILES_PER_GRAPH = N_PER // TILE_N      # 8
N_SLICE = N_PER // NCORES              # 125 (hypernet n-shard per core)
W1_COLS = NH * N_SLICE * D             # 24000
W2_COLS = NH * D * D                   # 12288
W3_COLS = NH * D                       # 192
H1_CHUNK_N = 5                         # n values per hypernet chunk
H1_CHUNK = H1_CHUNK_N * D              # 320 cols, <=512 psum limit
H1_CHUNKS_PER_HEAD = N_SLICE // H1_CHUNK_N   # 25
H1_DMA_GROUP = 5                       # matmul-chunks per DMA
G2_CHUNK_D = 8
G2_CHUNK = G2_CHUNK_D * D              # 512
G2_CHUNKS_PER_HEAD = D // G2_CHUNK_D   # 8

FP = mybir.dt.float32
AF = mybir.ActivationFunctionType
OP = mybir.AluOpType

_cache = {}


def _bcast_free(ap, n):
    """Append a broadcast (step 0) innermost free dim of size n."""
    return AP(ap.tensor, ap.offset, list(ap.ap) + [[0, n]])


def build_program(G, bg1v, bg2v, bg3v, stages=99):
    """Build the SPMD program (one NEFF, runs on all 8 cores)."""
    GCOLS16 = G // 16                   # idx cols per group
    GT = G // 128                       # edge tiles (matmuls) per group
    GCOLS = TILES_PER_CORE * GT         # dstl/norm cols per core

    nc = bacc.Bacc("TRN2", target_bir_lowering=False, debug=False,
                   enable_asserts=False, num_devices=NCORES)

    # ---- per-core inputs ----
    xT = nc.dram_tensor("xT", [F, NODES_PER_CORE], FP, kind="ExternalInput")
    idxw = nc.dram_tensor("idxw", [128, TILES_PER_CORE * GCOLS16], mybir.dt.int16,
                          kind="ExternalInput")
    dstl = nc.dram_tensor("dstl", [128, GCOLS], FP, kind="ExternalInput")
    normv = nc.dram_tensor("normv", [128, GCOLS], FP, kind="ExternalInput")
    Wc = [nc.dram_tensor(f"Wc{i}", [F, F], FP, kind="ExternalInput") for i in (1, 2, 3)]
    bc = [nc.dram_tensor(f"bc{i}", [F, 1], FP, kind="ExternalInput") for i in (1, 2, 3)]
    w1s = nc.dram_tensor("w1s", [F, W1_COLS], FP, kind="ExternalInput")
    Wg2 = nc.dram_tensor("Wg2", [F, W2_COLS], FP, kind="ExternalInput")
    Wg3 = nc.dram_tensor("Wg3", [F, W3_COLS], FP, kind="ExternalInput")
    acts = nc.dram_tensor("acts", [B, N_SLICE], FP, kind="ExternalInput")
    iota = nc.dram_tensor("iota", [128, TILE_N], FP, kind="ExternalInput")
    out = nc.dram_tensor("out", [B, 1], FP, kind="ExternalOutput")

    rg = [list(range(NCORES))]

    with tile.TileContext(nc) as tc:
        with tc.tile_pool(name="const", bufs=1) as cpool, \
             tc.tile_pool(name="meta", bufs=1) as mpool, \
             tc.tile_pool(name="msgs", bufs=4) as gpool, \
             tc.tile_pool(name="work", bufs=4) as wpool, \
             tc.tile_pool(name="hyp", bufs=3) as hpool, \
             tc.tile_pool(name="ps_agg", bufs=2, space="PSUM") as ps_agg, \
             tc.tile_pool(name="ps_feat", bufs=2, space="PSUM") as ps_feat, \
             tc.tile_pool(name="ps_hyp", bufs=2, space="PSUM") as ps_hyp, \
             tc.tile_pool(name="ps_misc", bufs=2, space="PSUM") as ps_misc, \
             tc.tile_pool(name="dram", bufs=1, space="DRAM") as dr:

            # ---- DRAM internal tensors (collective bounce buffers) ----
            ag_in = [dr.tile([NODES_PER_CORE, F], FP, tag=f"ag_in{i}",
                             name=f"ag_in{i}") for i in range(3)]
            tbl = [dr.tile([TOTAL, F], FP, addr_space="Shared", tag=f"tbl{i}",
                           name=f"tbl{i}") for i in range(3)]
            hg_in = dr.tile([GRAPHS_PER_CORE, F], FP, tag="hg_in")
            hg_out = dr.tile([B, F], FP, addr_space="Shared", tag="hg_out")
            h1_in = dr.tile([B, W3_COLS], FP, tag="h1_in")
            h1_out = dr.tile([B, W3_COLS], FP, addr_space="Shared", tag="h1_out")

            # ---- load constants ----
            Wc_sb = []
            bc_sb = []
            for i in range(3):
                w = cpool.tile([F, F], FP, tag=f"wc{i}")
                nc.sync.dma_start(out=w[:], in_=Wc[i][:])
                Wc_sb.append(w)
                b_ = cpool.tile([F, 1], FP, tag=f"bc{i}")
                nc.sync.dma_start(out=b_[:], in_=bc[i][:])
                bc_sb.append(b_)
            iota_sb = cpool.tile([128, TILE_N], FP, tag="iota")
            nc.sync.dma_start(out=iota_sb[:], in_=iota[:])
            bg1_sb = cpool.tile([128, 1], FP, tag="bg1c")
            nc.gpsimd.memset(bg1_sb[:], bg1v)
            bg2_sb = cpool.tile([128, 1], FP, tag="bg2c")
            nc.gpsimd.memset(bg2_sb[:], bg2v)
            bg3_sb = cpool.tile([128, 1], FP, tag="bg3c")
            nc.gpsimd.memset(bg3_sb[:], bg3v)
            eps_sb = cpool.tile([128, 1], FP, tag="epsc")
            nc.gpsimd.memset(eps_sb[:], EPS)
            ident = cpool.tile([128, 128], FP, tag="ident")
            make_identity(nc, ident[:])
            idxw_sb = mpool.tile([128, TILES_PER_CORE * GCOLS16], mybir.dt.int16)
            nc.sync.dma_start(out=idxw_sb[:], in_=idxw[:])
            dstl_sb = mpool.tile([128, GCOLS], FP)
            nc.sync.dma_start(out=dstl_sb[:], in_=dstl[:])
            normv_sb = mpool.tile([128, GCOLS], FP)
            nc.sync.dma_start(out=normv_sb[:], in_=normv[:])
            acts_sb = cpool.tile([B, N_SLICE], FP, tag="acts")
            nc.sync.dma_start(out=acts_sb[:], in_=acts[:])
            wg3_sb = cpool.tile([F, W3_COLS], FP, tag="wg3")
            nc.sync.dma_start(out=wg3_sb[:], in_=Wg3[:])
            xT_sb = mpool.tile([F, NODES_PER_CORE], FP)
            nc.sync.dma_start(out=xT_sb[:], in_=xT[:])

            # ---- phase 1: t1 = x @ Wc1 (own slice), all-gather ----
            for t in range(TILES_PER_CORE):
                ps = ps_feat.tile([TILE_N, F], FP, space="PSUM", tag="t_next")
                nc.tensor.matmul(ps[:], lhsT=xT_sb[:, t * TILE_N:(t + 1) * TILE_N],
                                 rhs=Wc_sb[0][:], start=True, stop=True)
                sb = wpool.tile([TILE_N, F], FP, tag="t_next_sb")
                nc.vector.tensor_copy(out=sb[:], in_=ps[:])
                nc.sync.dma_start(out=ag_in[0][t * TILE_N:(t + 1) * TILE_N, :], in_=sb[:])
            nc.gpsimd.collective_compute(
                "AllGather", OP.bypass, replica_groups=rg,
                ins=[ag_in[0][:]], outs=[tbl[0][:]])

            # ---- phases 2-4: GCN layers ----
            pool_cols = cpool.tile([128, TILES_PER_CORE], FP, tag="poolc")
            for layer in range(min(3, max(0, stages - 1))):
                src_tbl = tbl[layer]
                for g in range(TILES_PER_CORE):
                    msgs = gpool.tile([128, GT, F], FP, tag="msgs")
                    nc.gpsimd.dma_gather(
                        out_ap=msgs[:], in_ap=src_tbl[:],
                        idxs_ap=idxw_sb[:, g * GCOLS16:(g + 1) * GCOLS16],
                        num_idxs=G, num_idxs_reg=G, elem_size=F,
                        single_packet=False)
                    # selection matrices for the whole group in two DVE ops:
                    # st[e, j, d] = (iota[e, d] == dstl[e, gGT+j]) * norm[e, gGT+j]
                    st = wpool.tile([128, GT, TILE_N], FP, tag="st", bufs=2)
                    iota_bc = AP(iota_sb[:].tensor, iota_sb[:].offset,
                                 [iota_sb[:].ap[0], [0, GT], [1, TILE_N]])
                    nc.vector.tensor_tensor(
                        out=st[:], in0=iota_bc,
                        in1=_bcast_free(dstl_sb[:, g * GT:(g + 1) * GT], TILE_N),
                        op=OP.is_equal)
                    nc.vector.tensor_tensor(
                        out=st[:], in0=st[:],
                        in1=_bcast_free(normv_sb[:, g * GT:(g + 1) * GT], TILE_N),
                        op=OP.mult)
                    agg = ps_agg.tile([F, TILE_N], FP, space="PSUM", tag="agg")
                    for j in range(GT):
                        nc.tensor.matmul(agg[:], lhsT=msgs[:, j, :], rhs=st[:, j, :],
                                         start=(j == 0), stop=(j == GT - 1))
                    if layer < 2:
                        # hT = relu(aggT + b); t_next = hT.T @ Wc_next
                        hT = wpool.tile([F, TILE_N], FP, tag="hT")
                        nc.scalar.activation(out=hT[:], in_=agg[:], func=AF.Relu,
                                             bias=bc_sb[layer][:])
                        ps2 = ps_feat.tile([TILE_N, F], FP, space="PSUM",
                                           tag="t_next")
                        nc.tensor.matmul(ps2[:], lhsT=hT[:],
                                         rhs=Wc_sb[layer + 1][:],
                                         start=True, stop=True)
                        sb = wpool.tile([TILE_N, F], FP, tag="t_next_sb")
                        nc.vector.tensor_copy(out=sb[:], in_=ps2[:])
                        nc.sync.dma_start(
                            out=ag_in[layer + 1][g * TILE_N:(g + 1) * TILE_N, :],
                            in_=sb[:])
                    else:
                        # h3T = aggT + b; pool: sum over the 125 node columns
                        hT = wpool.tile([F, TILE_N], FP, tag="hT")
                        nc.scalar.activation(out=hT[:], in_=agg[:], func=AF.Identity,
                                             bias=bc_sb[2][:])
                        nc.vector.tensor_reduce(
                            out=pool_cols[:, g:g + 1], in_=hT[:],
                            axis=mybir.AxisListType.X, op=OP.add)
                if layer < 2:
                    nc.gpsimd.collective_compute(
                        "AllGather", OP.bypass, replica_groups=rg,
                        ins=[ag_in[layer + 1][:]], outs=[tbl[layer + 1][:]])

            if stages < 4:
                out_t = wpool.tile([B, 1], FP, tag="qmin")
                nc.gpsimd.memset(out_t[:], 0.0)
                nc.sync.dma_start(out=out[:], in_=out_t[:])
            do_rest = stages >= 4
            if do_rest:
                # ---- pooling -> h_g (mean over 1000 nodes), all-gather ----
                hgT_own = cpool.tile([128, 128], FP, tag="hgT_own")  # cols 0..3 used
                nc.gpsimd.memset(hgT_own[:], 0.0)
                for gi in range(GRAPHS_PER_CORE):
                    tmp = wpool.tile([128, 1], FP, tag="pool_tmp")
                    nc.vector.tensor_reduce(
                        out=tmp[:],
                        in_=pool_cols[:, gi * TILES_PER_GRAPH:(gi + 1) * TILES_PER_GRAPH],
                        axis=mybir.AxisListType.X, op=OP.add)
                    nc.scalar.activation(out=hgT_own[:, gi:gi + 1], in_=tmp[:],
                                         func=AF.Copy, scale=1.0 / N_PER)
                ps_tr = ps_misc.tile([128, 128], FP, space="PSUM", tag="tr")
                nc.tensor.transpose(out=ps_tr[:], in_=hgT_own[:], identity=ident[:])
                hg_sb = wpool.tile([GRAPHS_PER_CORE, F], FP, tag="hg_sb")
                nc.vector.tensor_copy(out=hg_sb[:], in_=ps_tr[0:GRAPHS_PER_CORE, :])
                nc.sync.dma_start(out=hg_in[:], in_=hg_sb[:])
                nc.gpsimd.collective_compute(
                    "AllGather", OP.bypass, replica_groups=rg,
                    ins=[hg_in[:]], outs=[hg_out[:]])

                # load full h_g and build h_gT [F, B] (lhsT for hypernet matmuls)
                hgf = wpool.tile([B, F], FP, tag="hgf")
                nc.sync.dma_start(out=hgf[:], in_=hg_out[:])
                ps_tr2 = ps_misc.tile([128, 128], FP, space="PSUM", tag="tr")
                nc.tensor.transpose(out=ps_tr2[:, 0:B], in_=hgf[:],
                                    identity=ident[0:B, 0:B])
                hgT = cpool.tile([F, B], FP, tag="hgT")
                nc.vector.tensor_copy(out=hgT[:], in_=ps_tr2[:, 0:B])

                # ---- phase 5: h1 partial (own n-slice of w1), all-reduce ----
                h1_acc = cpool.tile([B, W3_COLS], FP, tag="h1acc")
                nc.gpsimd.memset(h1_acc[:], 0.0)
                n_dma = H1_CHUNKS_PER_HEAD * NH // H1_DMA_GROUP  # 15
                for dm in range(n_dma):
                    wchunk = hpool.tile([F, H1_CHUNK * H1_DMA_GROUP], FP, tag="w1c")
                    c0 = dm * H1_CHUNK * H1_DMA_GROUP
                    nc.sync.dma_start(out=wchunk[:], in_=w1s[:, c0:c0 + H1_CHUNK * H1_DMA_GROUP])
                    for s in range(H1_DMA_GROUP):
                        cidx = dm * H1_DMA_GROUP + s          # global chunk index
                        h = cidx // H1_CHUNKS_PER_HEAD        # head
                        ci = cidx % H1_CHUNKS_PER_HEAD        # chunk within head
                        psg = ps_hyp.tile([B, H1_CHUNK], FP, space="PSUM", tag="psg")
                        nc.tensor.matmul(psg[:], lhsT=hgT[:],
                                         rhs=wchunk[:, s * H1_CHUNK:(s + 1) * H1_CHUNK],
                                         start=True, stop=True)
                        ge = hpool.tile([B, H1_CHUNK], FP, tag="ge")
                        nc.scalar.activation(out=ge[:], in_=psg[:], func=AF.Exp,
                                             bias=bg1_sb[0:B, :])
                        gsp = hpool.tile([B, H1_CHUNK], FP, tag="gsp")
                        nc.scalar.activation(out=gsp[:], in_=ge[:], func=AF.Ln, bias=1.0)
                        # multiply by actions (broadcast over d), reduce over n
                        n0 = ci * H1_CHUNK_N
                        a_b = _bcast_free(acts_sb[:, n0:n0 + H1_CHUNK_N], D)
                        gm = hpool.tile([B, H1_CHUNK_N, D], FP, tag="gm")
                        nc.vector.tensor_tensor(
                            out=gm[:], in0=gsp[:].rearrange("b (n d) -> b n d", d=D),
                            in1=a_b, op=OP.mult)
                        # reduce over n (outer free axis): view [B, D(step1), N(step D)]
                        gm_perm = AP(gm[:].tensor, gm[:].offset,
                                     [gm[:].ap[0], [1, D], [D, H1_CHUNK_N]])
                        red = hpool.tile([B, D], FP, tag="red")
                        nc.vector.tensor_reduce(out=red[:], in_=gm_perm,
                                                axis=mybir.AxisListType.X, op=OP.add)
                        nc.vector.tensor_tensor(
                            out=h1_acc[:, h * D:(h + 1) * D],
                            in0=h1_acc[:, h * D:(h + 1) * D], in1=red[:], op=OP.add)
                nc.sync.dma_start(out=h1_in[:], in_=h1_acc[:])
                nc.gpsimd.collective_compute(
                    "AllReduce", OP.add, replica_groups=rg,
                    ins=[h1_in[:]], outs=[h1_out[:]])

                # ---- phase 6: tail (all graphs, redundant on every core) ----
                h1f = wpool.tile([B, W3_COLS], FP, tag="h1f")
                nc.sync.dma_start(out=h1f[:], in_=h1_out[:])
                h1r = wpool.tile([B, W3_COLS], FP, tag="h1r")
                nc.scalar.activation(out=h1r[:], in_=h1f[:], func=AF.Relu)
                h1s = cpool.tile([B, W3_COLS], FP, tag="h1s")
                nc.scalar.activation(out=h1s[:], in_=h1r[:], func=AF.Sqrt, bias=eps_sb[0:B, :])

                h2_acc = cpool.tile([B, W3_COLS], FP, tag="h2acc")
                nc.gpsimd.memset(h2_acc[:], 0.0)
                n_g2 = NH * G2_CHUNKS_PER_HEAD  # 24
                for dm in range(n_g2 // 6):     # 4 DMAs of 6 chunks
                    wchunk = hpool.tile([F, G2_CHUNK * 6], FP, tag="w2c")
                    c0 = dm * G2_CHUNK * 6
                    nc.sync.dma_start(out=wchunk[:], in_=Wg2[:, c0:c0 + G2_CHUNK * 6])
                    for s in range(6):
                        cidx = dm * 6 + s
                        h = cidx // G2_CHUNKS_PER_HEAD
                        ci = cidx % G2_CHUNKS_PER_HEAD
                        psg = ps_hyp.tile([B, G2_CHUNK], FP, space="PSUM", tag="psg")
                        nc.tensor.matmul(psg[:], lhsT=hgT[:],
                                         rhs=wchunk[:, s * G2_CHUNK:(s + 1) * G2_CHUNK],
                                         start=True, stop=True)
                        ge = hpool.tile([B, G2_CHUNK], FP, tag="ge2")
                        nc.scalar.activation(out=ge[:], in_=psg[:], func=AF.Exp,
                                             bias=bg2_sb[0:B, :])
                        gsp = hpool.tile([B, G2_CHUNK], FP, tag="gsp2")
                        nc.scalar.activation(out=gsp[:], in_=ge[:], func=AF.Ln, bias=1.0)
                        d0 = ci * G2_CHUNK_D
                        hb = _bcast_free(h1s[:, h * D + d0:h * D + d0 + G2_CHUNK_D], D)
                        gm = hpool.tile([B, G2_CHUNK_D, D], FP, tag="gm2")
                        nc.vector.tensor_tensor(
                            out=gm[:], in0=gsp[:].rearrange("b (x e) -> b x e", e=D),
                            in1=hb, op=OP.mult)
                        gm_perm = AP(gm[:].tensor, gm[:].offset,
                                     [gm[:].ap[0], [1, D], [D, G2_CHUNK_D]])
                        red = hpool.tile([B, D], FP, tag="red2")
                        nc.vector.tensor_reduce(out=red[:], in_=gm_perm,
                                                axis=mybir.AxisListType.X, op=OP.add)
                        nc.vector.tensor_tensor(
                            out=h2_acc[:, h * D:(h + 1) * D],
                            in0=h2_acc[:, h * D:(h + 1) * D], in1=red[:], op=OP.add)
                # h2 = log1p(relu(h2_acc))
                h2r = wpool.tile([B, W3_COLS], FP, tag="h2r")
                nc.scalar.activation(out=h2r[:], in_=h2_acc[:], func=AF.Relu)
                h2l = wpool.tile([B, W3_COLS], FP, tag="h2l")
                nc.scalar.activation(out=h2l[:], in_=h2r[:], func=AF.Ln, bias=1.0)
                # w3 = softplus(hg @ Wg3 + bg3); q = sum_e h2*w3 per head; out = min_h
                psg3 = ps_hyp.tile([B, W3_COLS], FP, space="PSUM", tag="psg")
                nc.tensor.matmul(psg3[:], lhsT=hgT[:], rhs=wg3_sb[:], start=True, stop=True)
                g3e = wpool.tile([B, W3_COLS], FP, tag="g3e")
                nc.scalar.activation(out=g3e[:], in_=psg3[:], func=AF.Exp,
                                     bias=bg3_sb[0:B, :])
                g3 = wpool.tile([B, W3_COLS], FP, tag="g3")
                nc.scalar.activation(out=g3[:], in_=g3e[:], func=AF.Ln, bias=1.0)
                qm = wpool.tile([B, W3_COLS], FP, tag="qm")
                nc.vector.tensor_tensor(out=qm[:], in0=h2l[:], in1=g3[:], op=OP.mult)
                qh = wpool.tile([B, NH], FP, tag="qh")
                nc.vector.tensor_reduce(out=qh[:], in_=qm[:].rearrange("b (h e) -> b h e", e=D),
                                        axis=mybir.AxisListType.X, op=OP.add)
                qmin = wpool.tile([B, 1], FP, tag="qmin")
                nc.vector.tensor_reduce(out=qmin[:], in_=qh[:],
                                        axis=mybir.AxisListType.X, op=OP.min)
                nc.sync.dma_start(out=out[:], in_=qmin[:])

    nc.finalize()
    return nc


def _prepare(inputs):
    """Host-side preprocessing: norms, edge sort/bucket/pad, per-core arrays."""
    x = np.asarray(inputs["x"], np.float32)
    edge_index = np.asarray(inputs["edge_index"])
    batch_index = np.asarray(inputs["batch_index"])
    actions = np.asarray(inputs["actions"], np.float32)
    assert np.array_equal(batch_index.astype(np.int64),
                          np.repeat(np.arange(B, dtype=np.int64), N_PER)), \
        "kernel assumes contiguous equal-size graphs"

    src = np.concatenate([edge_index[0], np.arange(TOTAL, dtype=edge_index.dtype)])
    dst = np.concatenate([edge_index[1], np.arange(TOTAL, dtype=edge_index.dtype)])
    deg = np.bincount(dst, minlength=TOTAL).astype(np.float32)
    dinv = np.where(deg > 0, 1.0 / np.sqrt(deg), 0.0).astype(np.float32)
    norm = dinv[src] * dinv[dst]

    order = np.argsort(dst, kind="stable")
    src_s = src[order].astype(np.int32)
    dst_s = dst[order].astype(np.int32)
    norm_s = norm[order]

    n_groups = NCORES * TILES_PER_CORE
    # group of each edge = dst // TILE_N  (125-node tiles, globally numbered)
    grp = dst_s // TILE_N
    counts = np.bincount(grp, minlength=n_groups)
    G = int(np.ceil(counts.max() / 128) * 128)
    starts = np.zeros(n_groups + 1, np.int64)
    np.cumsum(counts, out=starts[1:])

    per_core = []
    GT = G // 128
    for k in range(NCORES):
        src_p = np.zeros((TILES_PER_CORE, G), np.int16)
        dstl_p = np.zeros((TILES_PER_CORE, G), np.float32)
        norm_p = np.zeros((TILES_PER_CORE, G), np.float32)
        for t in range(TILES_PER_CORE):
            gidx = k * TILES_PER_CORE + t
            s, e = starts[gidx], starts[gidx + 1]
            n = e - s
            src_p[t, :n] = src_s[s:e]
            dstl_p[t, :n] = (dst_s[s:e] - gidx * TILE_N).astype(np.float32)
            norm_p[t, :n] = norm_s[s:e]
        flat_src = src_p.reshape(-1)
        # dma_gather idx layout: [128, cols], row 16c+p col s -> idx[s*16+p], 8 Q7 replicas
        idxw = np.tile(flat_src.reshape(-1, 16).T, (8, 1)).copy()
        # dstl/norm layout: [128, cols], row p col c -> edge c*128+p
        dstl = dstl_p.reshape(-1, 128).T.copy()
        normv = norm_p.reshape(-1, 128).T.copy()
        per_core.append((idxw, dstl, normv))

    iota = np.tile(np.arange(TILE_N, dtype=np.float32), (128, 1)).copy()

    bg1 = np.asarray(inputs["bg1"], np.float32)
    bg2 = np.asarray(inputs["bg2"], np.float32)
    bg3 = np.asarray(inputs["bg3"], np.float32)
    assert np.all(bg1 == bg1.flat[0]) and np.all(bg2 == bg2.flat[0]) \
        and np.all(bg3 == bg3.flat[0]), "generator biases must be constant"

    w1_full = np.asarray(inputs["Wg1"], np.float32).reshape(F, NH, N_PER, D)

    in_maps = []
    for k in range(NCORES):
        idxw, dstl, normv = per_core[k]
        rows = slice(k * NODES_PER_CORE, (k + 1) * NODES_PER_CORE)
        w1s = np.ascontiguousarray(
            w1_full[:, :, k * N_SLICE:(k + 1) * N_SLICE, :].reshape(F, W1_COLS))
        in_maps.append(dict(
            xT=np.ascontiguousarray(x[rows].T),
            idxw=idxw, dstl=dstl, normv=normv,
            Wc1=np.asarray(inputs["Wc1"], np.float32),
            Wc2=np.asarray(inputs["Wc2"], np.float32),
            Wc3=np.asarray(inputs["Wc3"], np.float32),
            bc1=np.asarray(inputs["bc1"], np.float32).reshape(F, 1),
            bc2=np.asarray(inputs["bc2"], np.float32).reshape(F, 1),
            bc3=np.asarray(inputs["bc3"], np.float32).reshape(F, 1),
            w1s=w1s,
            Wg2=np.asarray(inputs["Wg2"], np.float32),
            Wg3=np.asarray(inputs["Wg3"], np.float32),
            acts=np.ascontiguousarray(actions[:, k * N_SLICE:(k + 1) * N_SLICE]),
            iota=iota,
        ))
    return G, float(bg1.flat[0]), float(bg2.flat[0]), float(bg3.flat[0]), in_maps


class _Runner:
    """Compiled SPMD executable reusable across calls (jit cache keyed here)."""

    def __init__(self, nc):
        import jax
        from concourse import bass2jax
        from jax.experimental.shard_map import shard_map
        from jax.sharding import Mesh, PartitionSpec

        bass2jax.install_neuronx_cc_hook()
        self.jax = jax
        part_name = nc.partition_id_tensor.name if nc.partition_id_tensor else None
        in_names, out_names, out_avals, zero_outs = [], [], [], []
        for alloc in nc.m.functions[0].allocations:
            if not isinstance(alloc, mybir.MemoryLocationSet):
                continue
            name = alloc.memorylocations[0].name
            if alloc.kind == "ExternalInput":
                if name != part_name:
                    in_names.append(name)
            elif alloc.kind == "ExternalOutput":
                out_names.append(name)
                shape = tuple(alloc.tensor_shape)
                dtype = mybir.dt.np(alloc.dtype)
                out_avals.append(jax.core.ShapedArray(shape, dtype))
                zero_outs.append(np.zeros(shape, dtype))
        self.in_names, self.out_names = in_names, out_names
        self.zero_outs = zero_outs
        n_params, n_outs = len(in_names), len(out_names)

        bind_names = in_names + out_names + ([part_name] if part_name else [])

        def _body(*args):
            operands = list(args)
            if part_name:
                operands.append(bass2jax.partition_id_tensor())
            outs = bass2jax._bass_exec_p.bind(
                *operands,
                out_avals=tuple(out_avals),
                in_names=tuple(bind_names),
                out_names=tuple(out_names),
                lowering_input_output_aliases=(),
                sim_require_finite=False,
                sim_require_nnan=False,
                nc=nc,
            )
            return tuple(outs)

        devices = jax.devices()[:NCORES]
        mesh = Mesh(np.asarray(devices), ("core",))
        self.mesh = mesh
        self.PartitionSpec = PartitionSpec
        self.fn = jax.jit(
            shard_map(_body, mesh=mesh,
                      in_specs=(PartitionSpec("core"),) * (n_params + n_outs),
                      out_specs=(PartitionSpec("core"),) * n_outs,
                      check_rep=False),
            donate_argnums=tuple(range(n_params, n_params + n_outs)),
            keep_unused=True)

    def concat_inputs(self, in_maps):
        return [np.concatenate([np.asarray(m[n]) for m in in_maps], axis=0)
                for n in self.in_names]

    def run(self, concat_in):
        zeros = [np.zeros((NCORES * z.shape[0], *z.shape[1:]), z.dtype)
                 for z in self.zero_outs]
        out_arrs = self.fn(*concat_in, *zeros)
        return [np.asarray(a) for a in out_arrs]


def _get_runner(G, bg1v, bg2v, bg3v):
    key = (G, bg1v, bg2v, bg3v)
    if key not in _cache:
        nc = build_program(G, bg1v, bg2v, bg3v)
        _cache[key] = _Runner(nc)
    return _cache[key]


def kernel(**inputs):
    G, bg1v, bg2v, bg3v, in_maps = _prepare(inputs)
    runner = _get_runner(G, bg1v, bg2v, bg3v)
    outs = runner.run(runner.concat_inputs(in_maps))
    # out tensor is [NCORES*B, 1]; every core computed the full [B] result
    return outs[0].reshape(NCORES, B)[0].astype(np.float32)



# revision 8
# speedup vs baseline: 7.1609x; 7.1609x over previous
"""Trainium2 Bass kernel for CDSQN (3-layer GCN + hypernetwork Q-head).

Contract: kernel(**inputs) takes the FULL unsharded inputs (numpy) and
returns the FULL [B] float32 output. Internally shards across 8
NeuronCores: nodes/edges by dst range (4000 nodes = 4 graphs per core),
GCN weights replicated, hypernetwork Wg1 sharded over the node axis.

Structure (v2):
  layer 1: gather raw-x messages (bf16) -> norm-weighted aggregate via
           selection-matrix matmuls -> Wc1 post-multiply (GCN linearity:
           A(xW) = (Ax)W) -> relu -> t2 = h1@Wc2 -> AllGather tbl2
  layer 2: gather tbl2 messages -> aggregate -> relu -> AllGather h2 tbl3
  layer 3 + mean-pool are fused into one linear op: pool_g depends
           linearly on h2, so pool = Wc3^T (tbl3^T c) / N + bc3 where
           c[v,g] = sum of edge norms from v into graph g (host-built).
           This removes the third gather pass and the h_g AllGather.
  hypernet: w1 n-slice partial h1 per core -> AllReduce -> tail.
All message tables / weight matmuls run in bf16 (fp32 PSUM accumulate).
"""
import sys

sys.path.insert(0, "/opt/trn_rl_repo")

import ml_dtypes
import numpy as np

import concourse.bacc as bacc
import concourse.bass as bass
import concourse.mybir as mybir
import concourse.tile as tile
from concourse.ap import AP
from concourse.masks import make_identity

# ---- problem constants (hardcoded per spec) ----
B = 32
N_PER = 1000
TOTAL = B * N_PER          # 32000
E = 512000
F = 128                    # node_feat_dim == hidden_dim
D = 64
NH = 3
EPS = 1e-6

NCORES = 8
NODES_PER_CORE = TOTAL // NCORES       # 4000
GRAPHS_PER_CORE = B // NCORES          # 4
TILE_N = 125                           # dst nodes per output tile
TILES_PER_CORE = NODES_PER_CORE // TILE_N   # 32
N_SLICE = N_PER // NCORES              # 125 (hypernet n-shard per core)
W1_COLS = NH * N_SLICE * D             # 24000
W2_COLS = NH * D * D                   # 12288
W3_COLS = NH * D                       # 192
H1_CHUNK_N = 5                         # n values per hypernet chunk
H1_CHUNK = H1_CHUNK_N * D              # 320 cols, <=512 psum limit
H1_CHUNKS_PER_HEAD = N_SLICE // H1_CHUNK_N   # 25
H1_DMA_GROUP = 5                       # matmul-chunks per DMA
G2_CHUNK_D = 8
G2_CHUNK = G2_CHUNK_D * D              # 512
G2_CHUNKS_PER_HEAD = D // G2_CHUNK_D   # 8
PTILES = TOTAL // 128                  # 250 pool tiles
PT_CHUNK = 10                          # pool tiles per DMA
BF16 = ml_dtypes.bfloat16

FP = mybir.dt.float32
BF = mybir.dt.bfloat16
AF = mybir.ActivationFunctionType
OP = mybir.AluOpType

_cache = {}


def _bcast_free(ap, n):
    """Append a broadcast (step 0) innermost free dim of size n."""
    return AP(ap.tensor, ap.offset, list(ap.ap) + [[0, n]])


def build_program(G, bg1v, bg2v, bg3v, stages=99):
    """Build the SPMD program (one NEFF, runs on all 8 cores)."""
    GCOLS16 = G // 16                   # idx cols per group
    GT = G // 128                       # edge tiles (matmuls) per group
    GCOLS = TILES_PER_CORE * GT         # dstl/norm cols per core

    nc = bacc.Bacc("TRN2", target_bir_lowering=False, debug=False,
                   enable_asserts=False, num_devices=NCORES)

    # ---- per-core inputs ----
    x_tbl = nc.dram_tensor("x_tbl", [TOTAL, F], BF, kind="ExternalInput")
    idxw = nc.dram_tensor("idxw", [128, TILES_PER_CORE * GCOLS16], mybir.dt.int16,
                          kind="ExternalInput")
    dstl = nc.dram_tensor("dstl", [128, GCOLS], BF, kind="ExternalInput")
    normv = nc.dram_tensor("normv", [128, GCOLS], BF, kind="ExternalInput")
    Wc = [nc.dram_tensor(f"Wc{i}", [F, F], BF, kind="ExternalInput") for i in (1, 2, 3)]
    bc = [nc.dram_tensor(f"bc{i}", [F, 1], FP, kind="ExternalInput") for i in (1, 2, 3)]
    cmat = nc.dram_tensor("cmat", [128, PTILES * B], BF, kind="ExternalInput")
    w1s = nc.dram_tensor("w1s", [F, W1_COLS], BF, kind="ExternalInput")
    Wg2 = nc.dram_tensor("Wg2", [F, W2_COLS], BF, kind="ExternalInput")
    Wg3 = nc.dram_tensor("Wg3", [F, W3_COLS], BF, kind="ExternalInput")
    acts = nc.dram_tensor("acts", [B, N_SLICE], FP, kind="ExternalInput")
    iota = nc.dram_tensor("iota", [128, TILE_N], BF, kind="ExternalInput")
    out = nc.dram_tensor("out", [B, 1], FP, kind="ExternalOutput")

    rg = [list(range(NCORES))]

    with tile.TileContext(nc) as tc:
        with tc.tile_pool(name="const", bufs=1) as cpool, \
             tc.tile_pool(name="meta", bufs=1) as mpool, \
             tc.tile_pool(name="msgs", bufs=4) as gpool, \
             tc.tile_pool(name="work", bufs=4) as wpool, \
             tc.tile_pool(name="hyp", bufs=3) as hpool, \
             tc.tile_pool(name="ps_agg", bufs=2, space="PSUM") as ps_agg, \
             tc.tile_pool(name="ps_feat", bufs=2, space="PSUM") as ps_feat, \
             tc.tile_pool(name="ps_hyp", bufs=2, space="PSUM") as ps_hyp, \
             tc.tile_pool(name="dram", bufs=1, space="DRAM") as dr:

            # ---- DRAM internal tensors (collective bounce buffers) ----
            ag2_in = dr.tile([NODES_PER_CORE, F], BF, tag="ag2_in", name="ag2_in")
            tbl2 = dr.tile([TOTAL, F], BF, addr_space="Shared", tag="tbl2",
                           name="tbl2")
            ag3_in = dr.tile([NODES_PER_CORE, F], BF, tag="ag3_in", name="ag3_in")
            tbl3 = dr.tile([TOTAL, F], BF, addr_space="Shared", tag="tbl3",
                           name="tbl3")
            h1_in = dr.tile([B, W3_COLS], FP, tag="h1_in")
            h1_out = dr.tile([B, W3_COLS], FP, addr_space="Shared", tag="h1_out")

            # ---- load constants ----
            Wc_sb = []
            bc_sb = []
            for i in range(3):
                w = cpool.tile([F, F], BF, tag=f"wc{i}")
                nc.sync.dma_start(out=w[:], in_=Wc[i][:])
                Wc_sb.append(w)
                b_ = cpool.tile([F, 1], FP, tag=f"bc{i}")
                nc.sync.dma_start(out=b_[:], in_=bc[i][:])
                bc_sb.append(b_)
            iota_sb = cpool.tile([128, TILE_N], BF, tag="iota")
            nc.sync.dma_start(out=iota_sb[:], in_=iota[:])
            bg1_sb = cpool.tile([128, 1], FP, tag="bg1c")
            nc.gpsimd.memset(bg1_sb[:], bg1v)
            bg2_sb = cpool.tile([128, 1], FP, tag="bg2c")
            nc.gpsimd.memset(bg2_sb[:], bg2v)
            bg3_sb = cpool.tile([128, 1], FP, tag="bg3c")
            nc.gpsimd.memset(bg3_sb[:], bg3v)
            eps_sb = cpool.tile([128, 1], FP, tag="epsc")
            nc.gpsimd.memset(eps_sb[:], EPS)
            ident_bf = cpool.tile([128, 128], BF, tag="identb")
            make_identity(nc, ident_bf[:])
            idxw_sb = mpool.tile([128, TILES_PER_CORE * GCOLS16], mybir.dt.int16)
            nc.sync.dma_start(out=idxw_sb[:], in_=idxw[:])
            dstl_sb = mpool.tile([128, GCOLS], BF)
            nc.sync.dma_start(out=dstl_sb[:], in_=dstl[:])
            normv_sb = mpool.tile([128, GCOLS], BF)
            nc.sync.dma_start(out=normv_sb[:], in_=normv[:])
            cmat_sb = mpool.tile([128, PTILES * B], BF)
            nc.sync.dma_start(out=cmat_sb[:], in_=cmat[:])
            acts_sb = cpool.tile([B, N_SLICE], FP, tag="acts")
            nc.sync.dma_start(out=acts_sb[:], in_=acts[:])
            wg3_sb = cpool.tile([F, W3_COLS], BF, tag="wg3")
            nc.sync.dma_start(out=wg3_sb[:], in_=Wg3[:])

            def gcn_group(src_tbl, g):
                """Gather + norm-weighted aggregate for dst tile g.
                Returns PSUM tile [F, TILE_N] = sum_e norm_e * msg_e."""
                msgs = gpool.tile([128, GT, F], BF, tag="msgs")
                nc.gpsimd.dma_gather(
                    out_ap=msgs[:], in_ap=src_tbl[:],
                    idxs_ap=idxw_sb[:, g * GCOLS16:(g + 1) * GCOLS16],
                    num_idxs=G, num_idxs_reg=G, elem_size=F,
                    single_packet=False)
                st = wpool.tile([128, GT, TILE_N], BF, tag="st", bufs=2)
                iota_bc = AP(iota_sb[:].tensor, iota_sb[:].offset,
                             [iota_sb[:].ap[0], [0, GT], [1, TILE_N]])
                nc.vector.tensor_tensor(
                    out=st[:], in0=iota_bc,
                    in1=_bcast_free(dstl_sb[:, g * GT:(g + 1) * GT], TILE_N),
                    op=OP.is_equal)
                nc.vector.tensor_tensor(
                    out=st[:], in0=st[:],
                    in1=_bcast_free(normv_sb[:, g * GT:(g + 1) * GT], TILE_N),
                    op=OP.mult)
                agg = ps_agg.tile([F, TILE_N], FP, space="PSUM", tag="agg")
                for j in range(GT):
                    nc.tensor.matmul(agg[:], lhsT=msgs[:, j, :], rhs=st[:, j, :],
                                     start=(j == 0), stop=(j == GT - 1))
                return agg

            # ---- layer 1: aggregate raw x, h1 = relu(Wc1^T agg + b1),
            #      t2 = h1 @ Wc2 (own nodes) ----
            if stages >= 1:
                for g in range(TILES_PER_CORE):
                    agg = gcn_group(x_tbl, g)
                    aggx = wpool.tile([F, TILE_N], BF, tag="aggx")
                    nc.vector.tensor_copy(out=aggx[:], in_=agg[:])
                    ps_h = ps_feat.tile([F, TILE_N], FP, space="PSUM", tag="psh")
                    nc.tensor.matmul(ps_h[:], lhsT=Wc_sb[0][:], rhs=aggx[:],
                                     start=True, stop=True)
                    hT = wpool.tile([F, TILE_N], BF, tag="hT")
                    nc.scalar.activation(out=hT[:], in_=ps_h[:], func=AF.Relu,
                                         bias=bc_sb[0][:])
                    ps_t2 = ps_feat.tile([TILE_N, F], FP, space="PSUM", tag="pst2")
                    nc.tensor.matmul(ps_t2[:], lhsT=hT[:], rhs=Wc_sb[1][:],
                                     start=True, stop=True)
                    t2sb = wpool.tile([TILE_N, F], BF, tag="t2sb")
                    nc.vector.tensor_copy(out=t2sb[:], in_=ps_t2[:])
                    nc.sync.dma_start(
                        out=ag2_in[g * TILE_N:(g + 1) * TILE_N, :], in_=t2sb[:])
            if stages >= 2:
                nc.gpsimd.collective_compute(
                    "AllGather", OP.bypass, replica_groups=rg,
                    ins=[ag2_in[:]], outs=[tbl2[:]])

            # ---- layer 2: aggregate t2, h2 = relu(agg + b2), all-gather h2 ----
            if stages >= 3:
                for g in range(TILES_PER_CORE):
                    agg = gcn_group(tbl2, g)
                    hT2 = wpool.tile([F, TILE_N], BF, tag="hT2")
                    nc.scalar.activation(out=hT2[:], in_=agg[:], func=AF.Relu,
                                         bias=bc_sb[1][:])
                    ps_tr = ps_feat.tile([TILE_N, F], BF, space="PSUM", tag="pst2")
                    nc.tensor.transpose(out=ps_tr[:], in_=hT2[:],
                                        identity=ident_bf[:])
                    h2sb = wpool.tile([TILE_N, F], BF, tag="h2sb")
                    nc.vector.tensor_copy(out=h2sb[:], in_=ps_tr[:])
                    nc.sync.dma_start(
                        out=ag3_in[g * TILE_N:(g + 1) * TILE_N, :], in_=h2sb[:])
            if stages >= 4:
                nc.gpsimd.collective_compute(
                    "AllGather", OP.bypass, replica_groups=rg,
                    ins=[ag3_in[:]], outs=[tbl3[:]])

            # ---- layer 3 + mean pool, fused linear:
            #      hgT = Wc3^T (tbl3^T cmat) / N_PER + bc3  [F, B] ----
            if stages >= 5:
                y_ps = ps_agg.tile([F, B], FP, space="PSUM", tag="agg")
                n_ch = PTILES // PT_CHUNK
                for ch in range(n_ch):
                    pt = gpool.tile([128, PT_CHUNK, F], BF, tag="ptile")
                    src = AP(tbl3[:].tensor, tbl3[:].offset + ch * PT_CHUNK * 128 * F,
                             [[F, 128], [128 * F, PT_CHUNK], [1, F]])
                    nc.sync.dma_start(out=pt[:], in_=src)
                    for tt in range(PT_CHUNK):
                        t = ch * PT_CHUNK + tt
                        nc.tensor.matmul(
                            y_ps[:], lhsT=pt[:, tt, :],
                            rhs=cmat_sb[:, t * B:(t + 1) * B],
                            start=(t == 0), stop=(t == PTILES - 1))
                y_sb = wpool.tile([F, B], BF, tag="ysb")
                nc.vector.tensor_copy(out=y_sb[:], in_=y_ps[:])
                hg_ps = ps_feat.tile([F, B], FP, space="PSUM", tag="psh")
                nc.tensor.matmul(hg_ps[:], lhsT=Wc_sb[2][:], rhs=y_sb[:],
                                 start=True, stop=True)
                hgT_bf = cpool.tile([F, B], BF, tag="hgTb")
                nc.scalar.activation(out=hgT_bf[:], in_=hg_ps[:], func=AF.Identity,
                                     bias=bc_sb[2][:], scale=1.0 / N_PER)

            if stages < 8:
                out_t = wpool.tile([B, 1], FP, tag="qmin")
                nc.gpsimd.memset(out_t[:], 0.0)
                nc.sync.dma_start(out=out[:], in_=out_t[:])

            # ---- hypernet h1 partial (own n-slice of w1), all-reduce ----
            if stages >= 6:
                h1_acc = cpool.tile([B, W3_COLS], FP, tag="h1acc")
                nc.gpsimd.memset(h1_acc[:], 0.0)
                n_dma = H1_CHUNKS_PER_HEAD * NH // H1_DMA_GROUP  # 15
                for dm in range(n_dma):
                    wchunk = hpool.tile([F, H1_CHUNK * H1_DMA_GROUP], BF, tag="w1c")
                    c0 = dm * H1_CHUNK * H1_DMA_GROUP
                    nc.sync.dma_start(out=wchunk[:],
                                      in_=w1s[:, c0:c0 + H1_CHUNK * H1_DMA_GROUP])
                    for s in range(H1_DMA_GROUP):
                        cidx = dm * H1_DMA_GROUP + s          # global chunk index
                        h = cidx // H1_CHUNKS_PER_HEAD        # head
                        ci = cidx % H1_CHUNKS_PER_HEAD        # chunk within head
                        psg = ps_hyp.tile([B, H1_CHUNK], FP, space="PSUM", tag="psg")
                        nc.tensor.matmul(psg[:], lhsT=hgT_bf[:],
                                         rhs=wchunk[:, s * H1_CHUNK:(s + 1) * H1_CHUNK],
                                         start=True, stop=True)
                        ge = hpool.tile([B, H1_CHUNK], FP, tag="ge")
                        nc.scalar.activation(out=ge[:], in_=psg[:], func=AF.Exp,
                                             bias=bg1_sb[0:B, :])
                        gsp = hpool.tile([B, H1_CHUNK], FP, tag="gsp")
                        nc.scalar.activation(out=gsp[:], in_=ge[:], func=AF.Ln, bias=1.0)
                        # multiply by actions (broadcast over d), reduce over n
                        n0 = ci * H1_CHUNK_N
                        a_b = _bcast_free(acts_sb[:, n0:n0 + H1_CHUNK_N], D)
                        gm = hpool.tile([B, H1_CHUNK_N, D], FP, tag="gm")
                        nc.vector.tensor_tensor(
                            out=gm[:], in0=gsp[:].rearrange("b (n d) -> b n d", d=D),
                            in1=a_b, op=OP.mult)
                        # reduce over n (outer free axis): view [B, D(step1), N(step D)]
                        gm_perm = AP(gm[:].tensor, gm[:].offset,
                                     [gm[:].ap[0], [1, D], [D, H1_CHUNK_N]])
                        red = hpool.tile([B, D], FP, tag="red")
                        nc.vector.tensor_reduce(out=red[:], in_=gm_perm,
                                                axis=mybir.AxisListType.X, op=OP.add)
                        nc.vector.tensor_tensor(
                            out=h1_acc[:, h * D:(h + 1) * D],
                            in0=h1_acc[:, h * D:(h + 1) * D], in1=red[:], op=OP.add)
                nc.sync.dma_start(out=h1_in[:], in_=h1_acc[:])
                nc.gpsimd.collective_compute(
                    "AllReduce", OP.add, replica_groups=rg,
                    ins=[h1_in[:]], outs=[h1_out[:]])

            # ---- tail (all graphs, redundant on every core) ----
            if stages >= 7:
                h1f = wpool.tile([B, W3_COLS], FP, tag="h1f")
                nc.sync.dma_start(out=h1f[:], in_=h1_out[:])
                h1r = wpool.tile([B, W3_COLS], FP, tag="h1r")
                nc.scalar.activation(out=h1r[:], in_=h1f[:], func=AF.Relu)
                h1s = cpool.tile([B, W3_COLS], FP, tag="h1s")
                nc.scalar.activation(out=h1s[:], in_=h1r[:], func=AF.Sqrt,
                                     bias=eps_sb[0:B, :])

                h2_acc = cpool.tile([B, W3_COLS], FP, tag="h2acc")
                nc.gpsimd.memset(h2_acc[:], 0.0)
                n_g2 = NH * G2_CHUNKS_PER_HEAD  # 24
                for dm in range(n_g2 // 6):     # 4 DMAs of 6 chunks
                    wchunk = hpool.tile([F, G2_CHUNK * 6], BF, tag="w2c")
                    c0 = dm * G2_CHUNK * 6
                    nc.sync.dma_start(out=wchunk[:], in_=Wg2[:, c0:c0 + G2_CHUNK * 6])
                    for s in range(6):
                        cidx = dm * 6 + s
                        h = cidx // G2_CHUNKS_PER_HEAD
                        ci = cidx % G2_CHUNKS_PER_HEAD
                        psg = ps_hyp.tile([B, G2_CHUNK], FP, space="PSUM", tag="psg")
                        nc.tensor.matmul(psg[:], lhsT=hgT_bf[:],
                                         rhs=wchunk[:, s * G2_CHUNK:(s + 1) * G2_CHUNK],
                                         start=True, stop=True)
                        ge = hpool.tile([B, G2_CHUNK], FP, tag="ge2")
                        nc.scalar.activation(out=ge[:], in_=psg[:], func=AF.Exp,
                                             bias=bg2_sb[0:B, :])
                        gsp = hpool.tile([B, G2_CHUNK], FP, tag="gsp2")
                        nc.scalar.activation(out=gsp[:], in_=ge[:], func=AF.Ln, bias=1.0)
                        d0 = ci * G2_CHUNK_D
                        hb = _bcast_free(h1s[:, h * D + d0:h * D + d0 + G2_CHUNK_D], D)
                        gm = hpool.tile([B, G2_CHUNK_D, D], FP, tag="gm2")
                        nc.vector.tensor_tensor(
                            out=gm[:], in0=gsp[:].rearrange("b (x e) -> b x e", e=D),
                            in1=hb, op=OP.mult)
                        gm_perm = AP(gm[:].tensor, gm[:].offset,
                                     [gm[:].ap[0], [1, D], [D, G2_CHUNK_D]])
                        red = hpool.tile([B, D], FP, tag="red2")
                        nc.vector.tensor_reduce(out=red[:], in_=gm_perm,
                                                axis=mybir.AxisListType.X, op=OP.add)
                        nc.vector.tensor_tensor(
                            out=h2_acc[:, h * D:(h + 1) * D],
                            in0=h2_acc[:, h * D:(h + 1) * D], in1=red[:], op=OP.add)
                # h2 = log1p(relu(h2_acc))
                h2r = wpool.tile([B, W3_COLS], FP, tag="h2r")
                nc.scalar.activation(out=h2r[:], in_=h2_acc[:], func=AF.Relu)
                h2l = wpool.tile([B, W3_COLS], FP, tag="h2l")
                nc.scalar.activation(out=h2l[:], in_=h2r[:], func=AF.Ln, bias=1.0)
                # w3 = softplus(hg @ Wg3 + bg3); q = sum_e h2*w3 per head; out = min_h
                psg3 = ps_hyp.tile([B, W3_COLS], FP, space="PSUM", tag="psg")
                nc.tensor.matmul(psg3[:], lhsT=hgT_bf[:], rhs=wg3_sb[:],
                                 start=True, stop=True)
                g3e = wpool.tile([B, W3_COLS], FP, tag="g3e")
                nc.scalar.activation(out=g3e[:], in_=psg3[:], func=AF.Exp,
                                     bias=bg3_sb[0:B, :])
                g3 = wpool.tile([B, W3_COLS], FP, tag="g3")
                nc.scalar.activation(out=g3[:], in_=g3e[:], func=AF.Ln, bias=1.0)
                qm = wpool.tile([B, W3_COLS], FP, tag="qm")
                nc.vector.tensor_tensor(out=qm[:], in0=h2l[:], in1=g3[:], op=OP.mult)
                qh = wpool.tile([B, NH], FP, tag="qh")
                nc.vector.tensor_reduce(out=qh[:],
                                        in_=qm[:].rearrange("b (h e) -> b h e", e=D),
                                        axis=mybir.AxisListType.X, op=OP.add)
                qmin = wpool.tile([B, 1], FP, tag="qmin")
                nc.vector.tensor_reduce(out=qmin[:], in_=qh[:],
                                        axis=mybir.AxisListType.X, op=OP.min)
                nc.sync.dma_start(out=out[:], in_=qmin[:])

    nc.finalize()
    return nc


def _prepare(inputs):
    """Host-side preprocessing: norms, edge sort/bucket/pad, per-core arrays."""
    x = np.asarray(inputs["x"], np.float32)
    edge_index = np.asarray(inputs["edge_index"])
    batch_index = np.asarray(inputs["batch_index"])
    actions = np.asarray(inputs["actions"], np.float32)
    assert np.array_equal(batch_index.astype(np.int64),
                          np.repeat(np.arange(B, dtype=np.int64), N_PER)), \
        "kernel assumes contiguous equal-size graphs"

    src = np.concatenate([edge_index[0], np.arange(TOTAL, dtype=edge_index.dtype)])
    dst = np.concatenate([edge_index[1], np.arange(TOTAL, dtype=edge_index.dtype)])
    deg = np.bincount(dst, minlength=TOTAL).astype(np.float32)
    dinv = np.where(deg > 0, 1.0 / np.sqrt(deg), 0.0).astype(np.float32)
    norm = dinv[src] * dinv[dst]

    order = np.argsort(dst, kind="stable")
    src_s = src[order].astype(np.int32)
    dst_s = dst[order].astype(np.int32)
    norm_s = norm[order]

    n_groups = NCORES * TILES_PER_CORE
    # group of each edge = dst // TILE_N  (125-node tiles, globally numbered)
    grp = dst_s // TILE_N
    counts = np.bincount(grp, minlength=n_groups)
    G = int(np.ceil(counts.max() / 128) * 128)
    starts = np.zeros(n_groups + 1, np.int64)
    np.cumsum(counts, out=starts[1:])

    per_core = []
    for k in range(NCORES):
        src_p = np.zeros((TILES_PER_CORE, G), np.int16)
        dstl_p = np.zeros((TILES_PER_CORE, G), np.float32)
        norm_p = np.zeros((TILES_PER_CORE, G), np.float32)
        for t in range(TILES_PER_CORE):
            gidx = k * TILES_PER_CORE + t
            s, e = starts[gidx], starts[gidx + 1]
            n = e - s
            src_p[t, :n] = src_s[s:e]
            dstl_p[t, :n] = (dst_s[s:e] - gidx * TILE_N).astype(np.float32)
            norm_p[t, :n] = norm_s[s:e]
        flat_src = src_p.reshape(-1)
        # dma_gather idx layout: [128, cols], row 16c+p col s -> idx[s*16+p], 8 Q7 replicas
        idxw = np.tile(flat_src.reshape(-1, 16).T, (8, 1)).copy()
        # dstl/norm layout: [128, cols], row p col c -> edge c*128+p
        dstl = dstl_p.reshape(-1, 128).T.astype(BF16).copy()
        normv = norm_p.reshape(-1, 128).T.astype(BF16).copy()
        per_core.append((idxw, dstl, normv))

    iota = np.tile(np.arange(TILE_N, dtype=np.float32), (128, 1)).astype(BF16)

    # pool weights: c[v, g] = sum of norm over edges v -> graph g (incl loops)
    gof = src.astype(np.int64) * B + dst.astype(np.int64) // N_PER
    c_full = np.bincount(gof, weights=norm.astype(np.float64),
                         minlength=TOTAL * B).reshape(TOTAL, B)
    cmat = (c_full.reshape(PTILES, 128, B).transpose(1, 0, 2)
            .reshape(128, PTILES * B).astype(BF16).copy())

    bg1 = np.asarray(inputs["bg1"], np.float32)
    bg2 = np.asarray(inputs["bg2"], np.float32)
    bg3 = np.asarray(inputs["bg3"], np.float32)
    assert np.all(bg1 == bg1.flat[0]) and np.all(bg2 == bg2.flat[0]) \
        and np.all(bg3 == bg3.flat[0]), "generator biases must be constant"

    x_tbl = np.ascontiguousarray(x.astype(BF16))
    w1_full = np.asarray(inputs["Wg1"], np.float32).reshape(F, NH, N_PER, D)
    Wc1 = np.asarray(inputs["Wc1"], np.float32).astype(BF16)
    Wc2 = np.asarray(inputs["Wc2"], np.float32).astype(BF16)
    Wc3 = np.asarray(inputs["Wc3"], np.float32).astype(BF16)
    Wg2b = np.asarray(inputs["Wg2"], np.float32).astype(BF16)
    Wg3b = np.asarray(inputs["Wg3"], np.float32).astype(BF16)

    in_maps = []
    for k in range(NCORES):
        idxw, dstl, normv = per_core[k]
        w1s = np.ascontiguousarray(
            w1_full[:, :, k * N_SLICE:(k + 1) * N_SLICE, :]
            .reshape(F, W1_COLS)).astype(BF16)
        in_maps.append(dict(
            x_tbl=x_tbl,
            idxw=idxw, dstl=dstl, normv=normv,
            Wc1=Wc1, Wc2=Wc2, Wc3=Wc3,
            bc1=np.asarray(inputs["bc1"], np.float32).reshape(F, 1),
            bc2=np.asarray(inputs["bc2"], np.float32).reshape(F, 1),
            bc3=np.asarray(inputs["bc3"], np.float32).reshape(F, 1),
            cmat=cmat,
            w1s=w1s,
            Wg2=Wg2b,
            Wg3=Wg3b,
            acts=np.ascontiguousarray(actions[:, k * N_SLICE:(k + 1) * N_SLICE]),
            iota=iota,
        ))
    return G, float(bg1.flat[0]), float(bg2.flat[0]), float(bg3.flat[0]), in_maps


class _Runner:
    """Compiled SPMD executable reusable across calls (jit cache keyed here)."""

    def __init__(self, nc):
        import jax
        from concourse import bass2jax
        from jax.experimental.shard_map import shard_map
        from jax.sharding import Mesh, PartitionSpec

        bass2jax.install_neuronx_cc_hook()
        self.jax = jax
        part_name = nc.partition_id_tensor.name if nc.partition_id_tensor else None
        in_names, out_names, out_avals, zero_outs = [], [], [], []
        for alloc in nc.m.functions[0].allocations:
            if not isinstance(alloc, mybir.MemoryLocationSet):
                continue
            name = alloc.memorylocations[0].name
            if alloc.kind == "ExternalInput":
                if name != part_name:
                    in_names.append(name)
            elif alloc.kind == "ExternalOutput":
                out_names.append(name)
                shape = tuple(alloc.tensor_shape)
                dtype = mybir.dt.np(alloc.dtype)
                out_avals.append(jax.core.ShapedArray(shape, dtype))
                zero_outs.append(np.zeros(shape, dtype))
        self.in_names, self.out_names = in_names, out_names
        self.zero_outs = zero_outs
        n_params, n_outs = len(in_names), len(out_names)

        bind_names = in_names + out_names + ([part_name] if part_name else [])

        def _body(*args):
            operands = list(args)
            if part_name:
                operands.append(bass2jax.partition_id_tensor())
            outs = bass2jax._bass_exec_p.bind(
                *operands,
                out_avals=tuple(out_avals),
                in_names=tuple(bind_names),
                out_names=tuple(out_names),
                lowering_input_output_aliases=(),
                sim_require_finite=False,
                sim_require_nnan=False,
                nc=nc,
            )
            return tuple(outs)

        devices = jax.devices()[:NCORES]
        mesh = Mesh(np.asarray(devices), ("core",))
        self.mesh = mesh
        self.PartitionSpec = PartitionSpec
        self.fn = jax.jit(
            shard_map(_body, mesh=mesh,
                      in_specs=(PartitionSpec("core"),) * (n_params + n_outs),
                      out_specs=(PartitionSpec("core"),) * n_outs,
                      check_rep=False),
            donate_argnums=tuple(range(n_params, n_params + n_outs)),
            keep_unused=True)

    def concat_inputs(self, in_maps):
        return [np.concatenate([np.asarray(m[n]) for m in in_maps], axis=0)
                for n in self.in_names]

    def run(self, concat_in):
        zeros = [np.zeros((NCORES * z.shape[0], *z.shape[1:]), z.dtype)
                 for z in self.zero_outs]
        out_arrs = self.fn(*concat_in, *zeros)
        return [np.asarray(a) for a in out_arrs]


def _get_runner(G, bg1v, bg2v, bg3v):
    key = (G, bg1v, bg2v, bg3v)
    if key not in _cache:
        nc = build_program(G, bg1v, bg2v, bg3v)
        _cache[key] = _Runner(nc)
    return _cache[key]


def kernel(**inputs):
    G, bg1v, bg2v, bg3v, in_maps = _prepare(inputs)
    runner = _get_runner(G, bg1v, bg2v, bg3v)
    outs = runner.run(runner.concat_inputs(in_maps))
    # out tensor is [NCORES*B, 1]; every core computed the full [B] result
    return outs[0].reshape(NCORES, B)[0].astype(np.float32)


# revision 10
# speedup vs baseline: 7.2577x; 1.0135x over previous
"""Trainium2 Bass kernel for CDSQN (3-layer GCN + hypernetwork Q-head).

Contract: kernel(**inputs) takes the FULL unsharded inputs (numpy) and
returns the FULL [B] float32 output. Internally shards across 8
NeuronCores: nodes/edges by dst range (4000 nodes = 4 graphs per core),
GCN weights replicated, hypernetwork Wg1 sharded over the node axis.

Structure (v2):
  layer 1: gather raw-x messages (bf16) -> norm-weighted aggregate via
           selection-matrix matmuls -> Wc1 post-multiply (GCN linearity:
           A(xW) = (Ax)W) -> relu -> t2 = h1@Wc2 -> AllGather tbl2
  layer 2: gather tbl2 messages -> aggregate -> relu -> AllGather h2 tbl3
  layer 3 + mean-pool are fused into one linear op: pool_g depends
           linearly on h2, so pool = Wc3^T (tbl3^T c) / N + bc3 where
           c[v,g] = sum of edge norms from v into graph g (host-built).
           This removes the third gather pass and the h_g AllGather.
  hypernet: w1 n-slice partial h1 per core -> AllReduce -> tail.
All message tables / weight matmuls run in bf16 (fp32 PSUM accumulate).
"""
import sys

sys.path.insert(0, "/opt/trn_rl_repo")

import ml_dtypes
import numpy as np

import concourse.bacc as bacc
import concourse.bass as bass
import concourse.mybir as mybir
import concourse.tile as tile
from concourse.ap import AP
from concourse.masks import make_identity

# ---- problem constants (hardcoded per spec) ----
B = 32
N_PER = 1000
TOTAL = B * N_PER          # 32000
E = 512000
F = 128                    # node_feat_dim == hidden_dim
D = 64
NH = 3
EPS = 1e-6

NCORES = 8
NODES_PER_CORE = TOTAL // NCORES       # 4000
GRAPHS_PER_CORE = B // NCORES          # 4
TILE_N = 125                           # dst nodes per output tile
TILES_PER_CORE = NODES_PER_CORE // TILE_N   # 32
N_SLICE = N_PER // NCORES              # 125 (hypernet n-shard per core)
W1_COLS = NH * N_SLICE * D             # 24000
W2_COLS = NH * D * D                   # 12288
W3_COLS = NH * D                       # 192
H1_CHUNK_N = 5                         # n values per hypernet chunk
H1_CHUNK = H1_CHUNK_N * D              # 320 cols, <=512 psum limit
H1_CHUNKS_PER_HEAD = N_SLICE // H1_CHUNK_N   # 25
H1_DMA_GROUP = 5                       # matmul-chunks per DMA
G2_CHUNK_D = 8
G2_CHUNK = G2_CHUNK_D * D              # 512
G2_CHUNKS_PER_HEAD = D // G2_CHUNK_D   # 8
PTILES = TOTAL // 128                  # 250 pool tiles
PT_CHUNK = 10                          # pool tiles per DMA
BF16 = ml_dtypes.bfloat16

FP = mybir.dt.float32
BF = mybir.dt.bfloat16
AF = mybir.ActivationFunctionType
OP = mybir.AluOpType

_cache = {}


def _bcast_free(ap, n):
    """Append a broadcast (step 0) innermost free dim of size n."""
    return AP(ap.tensor, ap.offset, list(ap.ap) + [[0, n]])


def build_program(G, bg1v, bg2v, bg3v, stages=99):
    """Build the SPMD program (one NEFF, runs on all 8 cores)."""
    GCOLS16 = G // 16                   # idx cols per group
    GT = G // 128                       # edge tiles (matmuls) per group
    GCOLS = TILES_PER_CORE * GT         # dstl/norm cols per core

    nc = bacc.Bacc("TRN2", target_bir_lowering=False, debug=False,
                   enable_asserts=False, num_devices=NCORES)

    # ---- per-core inputs ----
    x_tbl = nc.dram_tensor("x_tbl", [TOTAL, F], BF, kind="ExternalInput")
    idxw = nc.dram_tensor("idxw", [128, TILES_PER_CORE * GCOLS16], mybir.dt.int16,
                          kind="ExternalInput")
    dstl = nc.dram_tensor("dstl", [128, GCOLS], BF, kind="ExternalInput")
    normv = nc.dram_tensor("normv", [128, GCOLS], BF, kind="ExternalInput")
    Wc = [nc.dram_tensor(f"Wc{i}", [F, F], BF, kind="ExternalInput") for i in (1, 2, 3)]
    bc = [nc.dram_tensor(f"bc{i}", [F, 1], FP, kind="ExternalInput") for i in (1, 2, 3)]
    cown = nc.dram_tensor("cown", [128, TILES_PER_CORE * B], BF,
                          kind="ExternalInput")
    bc2r = nc.dram_tensor("bc2r", [1, F], BF, kind="ExternalInput")
    w1s = nc.dram_tensor("w1s", [F, W1_COLS], BF, kind="ExternalInput")
    Wg2 = nc.dram_tensor("Wg2", [F, W2_COLS], BF, kind="ExternalInput")
    Wg3 = nc.dram_tensor("Wg3", [F, W3_COLS], BF, kind="ExternalInput")
    acts = nc.dram_tensor("acts", [B, N_SLICE], FP, kind="ExternalInput")
    iota = nc.dram_tensor("iota", [128, TILE_N], BF, kind="ExternalInput")
    out = nc.dram_tensor("out", [B, 1], FP, kind="ExternalOutput")

    rg = [list(range(NCORES))]

    with tile.TileContext(nc) as tc:
        with tc.tile_pool(name="const", bufs=1) as cpool, \
             tc.tile_pool(name="meta", bufs=1) as mpool, \
             tc.tile_pool(name="msgs", bufs=4) as gpool, \
             tc.tile_pool(name="work", bufs=4) as wpool, \
             tc.tile_pool(name="hyp", bufs=3) as hpool, \
             tc.tile_pool(name="ps_agg", bufs=2, space="PSUM") as ps_agg, \
             tc.tile_pool(name="ps_feat", bufs=2, space="PSUM") as ps_feat, \
             tc.tile_pool(name="ps_hyp", bufs=2, space="PSUM") as ps_hyp, \
             tc.tile_pool(name="dram", bufs=1, space="DRAM") as dr:

            # ---- DRAM internal tensors (collective bounce buffers) ----
            ag2_in = dr.tile([NODES_PER_CORE, F], BF, tag="ag2_in", name="ag2_in")
            tbl2 = dr.tile([TOTAL, F], BF, addr_space="Shared", tag="tbl2",
                           name="tbl2")
            y_in = dr.tile([F, B], FP, tag="y_in")
            y_out = dr.tile([F, B], FP, addr_space="Shared", tag="y_out")
            h1_in = dr.tile([B, W3_COLS], FP, tag="h1_in")
            h1_out = dr.tile([B, W3_COLS], FP, addr_space="Shared", tag="h1_out")

            # ---- load constants ----
            Wc_sb = []
            bc_sb = []
            for i in range(3):
                w = cpool.tile([F, F], BF, tag=f"wc{i}")
                nc.sync.dma_start(out=w[:], in_=Wc[i][:])
                Wc_sb.append(w)
                b_ = cpool.tile([F, 1], FP, tag=f"bc{i}")
                nc.sync.dma_start(out=b_[:], in_=bc[i][:])
                bc_sb.append(b_)
            iota_sb = cpool.tile([128, TILE_N], BF, tag="iota")
            nc.sync.dma_start(out=iota_sb[:], in_=iota[:])
            bg1_sb = cpool.tile([128, 1], FP, tag="bg1c")
            nc.gpsimd.memset(bg1_sb[:], bg1v)
            bg2_sb = cpool.tile([128, 1], FP, tag="bg2c")
            nc.gpsimd.memset(bg2_sb[:], bg2v)
            bg3_sb = cpool.tile([128, 1], FP, tag="bg3c")
            nc.gpsimd.memset(bg3_sb[:], bg3v)
            eps_sb = cpool.tile([128, 1], FP, tag="epsc")
            nc.gpsimd.memset(eps_sb[:], EPS)
            ones_sb = cpool.tile([1, TILE_N], BF, tag="ones1")
            nc.gpsimd.memset(ones_sb[:], 1.0)
            bc2r_sb = cpool.tile([1, F], BF, tag="bc2r")
            nc.sync.dma_start(out=bc2r_sb[:], in_=bc2r[:])
            idxw_sb = mpool.tile([128, TILES_PER_CORE * GCOLS16], mybir.dt.int16)
            nc.sync.dma_start(out=idxw_sb[:], in_=idxw[:])
            dstl_sb = mpool.tile([128, GCOLS], BF)
            nc.sync.dma_start(out=dstl_sb[:], in_=dstl[:])
            normv_sb = mpool.tile([128, GCOLS], BF)
            nc.sync.dma_start(out=normv_sb[:], in_=normv[:])
            cown_sb = mpool.tile([128, TILES_PER_CORE * B], BF)
            nc.sync.dma_start(out=cown_sb[:], in_=cown[:])
            acts_sb = cpool.tile([B, N_SLICE], FP, tag="acts")
            nc.sync.dma_start(out=acts_sb[:], in_=acts[:])
            wg3_sb = cpool.tile([F, W3_COLS], BF, tag="wg3")
            nc.sync.dma_start(out=wg3_sb[:], in_=Wg3[:])

            def gcn_group(src_tbl, g, node_major=False):
                """Gather + norm-weighted aggregate for dst tile g.
                Feature-major: PSUM [F, TILE_N]. Node-major: PSUM
                [TILE_N, F] with bc2 bias folded in via a rank-1 matmul."""
                msgs = gpool.tile([128, GT, F], BF, tag="msgs")
                nc.gpsimd.dma_gather(
                    out_ap=msgs[:], in_ap=src_tbl[:],
                    idxs_ap=idxw_sb[:, g * GCOLS16:(g + 1) * GCOLS16],
                    num_idxs=G, num_idxs_reg=G, elem_size=F,
                    single_packet=False)
                st = wpool.tile([128, GT, TILE_N], BF, tag="st", bufs=2)
                iota_bc = AP(iota_sb[:].tensor, iota_sb[:].offset,
                             [iota_sb[:].ap[0], [0, GT], [1, TILE_N]])
                nc.vector.tensor_tensor(
                    out=st[:], in0=iota_bc,
                    in1=_bcast_free(dstl_sb[:, g * GT:(g + 1) * GT], TILE_N),
                    op=OP.is_equal)
                nc.vector.tensor_tensor(
                    out=st[:], in0=st[:],
                    in1=_bcast_free(normv_sb[:, g * GT:(g + 1) * GT], TILE_N),
                    op=OP.mult)
                if node_major:
                    agg = ps_agg.tile([TILE_N, F], FP, space="PSUM", tag="agg")
                    for j in range(GT):
                        nc.tensor.matmul(agg[:], lhsT=st[:, j, :], rhs=msgs[:, j, :],
                                         start=(j == 0), stop=False)
                    nc.tensor.matmul(agg[:], lhsT=ones_sb[:], rhs=bc2r_sb[:],
                                     start=False, stop=True)
                else:
                    agg = ps_agg.tile([F, TILE_N], FP, space="PSUM", tag="agg")
                    for j in range(GT):
                        nc.tensor.matmul(agg[:], lhsT=msgs[:, j, :], rhs=st[:, j, :],
                                         start=(j == 0), stop=(j == GT - 1))
                return agg

            # ---- layer 1: aggregate raw x, h1 = relu(Wc1^T agg + b1),
            #      t2 = h1 @ Wc2 (own nodes) ----
            if stages >= 1:
                for g in range(TILES_PER_CORE):
                    agg = gcn_group(x_tbl, g)
                    aggx = wpool.tile([F, TILE_N], BF, tag="aggx")
                    nc.vector.tensor_copy(out=aggx[:], in_=agg[:])
                    ps_h = ps_feat.tile([F, TILE_N], FP, space="PSUM", tag="feat")
                    nc.tensor.matmul(ps_h[:], lhsT=Wc_sb[0][:], rhs=aggx[:],
                                     start=True, stop=True)
                    hT = wpool.tile([F, TILE_N], BF, tag="hT")
                    nc.scalar.activation(out=hT[:], in_=ps_h[:], func=AF.Relu,
                                         bias=bc_sb[0][:])
                    ps_t2 = ps_feat.tile([TILE_N, F], FP, space="PSUM", tag="feat")
                    nc.tensor.matmul(ps_t2[:], lhsT=hT[:], rhs=Wc_sb[1][:],
                                     start=True, stop=True)
                    t2sb = wpool.tile([TILE_N, F], BF, tag="t2sb")
                    nc.vector.tensor_copy(out=t2sb[:], in_=ps_t2[:])
                    nc.sync.dma_start(
                        out=ag2_in[g * TILE_N:(g + 1) * TILE_N, :], in_=t2sb[:])
            if stages >= 2:
                nc.gpsimd.collective_compute(
                    "AllGather", OP.bypass, replica_groups=rg,
                    ins=[ag2_in[:]], outs=[tbl2[:]])

            # ---- layer 2: node-major agg, h2 = relu(agg + b2), local pool
            #      partial y += h2^T c_own accumulated in PSUM ----
            if stages >= 3:
                y_ps = ps_agg.tile([F, B], FP, space="PSUM", tag="ypool", bufs=1)
                for g in range(TILES_PER_CORE):
                    agg = gcn_group(tbl2, g, node_major=True)
                    h2sb = wpool.tile([TILE_N, F], BF, tag="h2sb")
                    nc.scalar.activation(out=h2sb[:], in_=agg[:], func=AF.Relu)
                    nc.tensor.matmul(y_ps[:], lhsT=h2sb[:],
                                     rhs=cown_sb[0:TILE_N, g * B:(g + 1) * B],
                                     start=(g == 0), stop=(g == TILES_PER_CORE - 1))
            if stages >= 4:
                y_sb = wpool.tile([F, B], FP, tag="ysb")
                nc.vector.tensor_copy(out=y_sb[:], in_=y_ps[:])
                nc.sync.dma_start(out=y_in[:], in_=y_sb[:])
                nc.gpsimd.collective_compute(
                    "AllReduce", OP.add, replica_groups=rg,
                    ins=[y_in[:]], outs=[y_out[:]])

            # ---- layer 3 + mean pool tail: hgT = Wc3^T y / N_PER + bc3 ----
            if stages >= 5:
                y_f = wpool.tile([F, B], FP, tag="yf")
                nc.sync.dma_start(out=y_f[:], in_=y_out[:])
                y_bf = wpool.tile([F, B], BF, tag="ybf")
                nc.vector.tensor_copy(out=y_bf[:], in_=y_f[:])
                hg_ps = ps_feat.tile([F, B], FP, space="PSUM", tag="feat")
                nc.tensor.matmul(hg_ps[:], lhsT=Wc_sb[2][:], rhs=y_bf[:],
                                 start=True, stop=True)
                hgT_bf = cpool.tile([F, B], BF, tag="hgTb")
                nc.scalar.activation(out=hgT_bf[:], in_=hg_ps[:], func=AF.Identity,
                                     bias=bc_sb[2][:], scale=1.0 / N_PER)

            if stages < 8:
                out_t = wpool.tile([B, 1], FP, tag="qmin")
                nc.gpsimd.memset(out_t[:], 0.0)
                nc.sync.dma_start(out=out[:], in_=out_t[:])

            # ---- hypernet h1 partial (own n-slice of w1), all-reduce ----
            if stages >= 6:
                h1_acc = cpool.tile([B, W3_COLS], FP, tag="h1acc")
                nc.gpsimd.memset(h1_acc[:], 0.0)
                n_dma = H1_CHUNKS_PER_HEAD * NH // H1_DMA_GROUP  # 15
                for dm in range(n_dma):
                    wchunk = hpool.tile([F, H1_CHUNK * H1_DMA_GROUP], BF, tag="w1c")
                    c0 = dm * H1_CHUNK * H1_DMA_GROUP
                    nc.sync.dma_start(out=wchunk[:],
                                      in_=w1s[:, c0:c0 + H1_CHUNK * H1_DMA_GROUP])
                    for s in range(H1_DMA_GROUP):
                        cidx = dm * H1_DMA_GROUP + s          # global chunk index
                        h = cidx // H1_CHUNKS_PER_HEAD        # head
                        ci = cidx % H1_CHUNKS_PER_HEAD        # chunk within head
                        psg = ps_hyp.tile([B, H1_CHUNK], FP, space="PSUM", tag="psg")
                        nc.tensor.matmul(psg[:], lhsT=hgT_bf[:],
                                         rhs=wchunk[:, s * H1_CHUNK:(s + 1) * H1_CHUNK],
                                         start=True, stop=True)
                        ge = hpool.tile([B, H1_CHUNK], FP, tag="ge")
                        nc.scalar.activation(out=ge[:], in_=psg[:], func=AF.Exp,
                                             bias=bg1_sb[0:B, :])
                        gsp = hpool.tile([B, H1_CHUNK], FP, tag="gsp")
                        nc.scalar.activation(out=gsp[:], in_=ge[:], func=AF.Ln, bias=1.0)
                        # multiply by actions (broadcast over d), reduce over n
                        n0 = ci * H1_CHUNK_N
                        a_b = _bcast_free(acts_sb[:, n0:n0 + H1_CHUNK_N], D)
                        gm = hpool.tile([B, H1_CHUNK_N, D], FP, tag="gm")
                        nc.vector.tensor_tensor(
                            out=gm[:], in0=gsp[:].rearrange("b (n d) -> b n d", d=D),
                            in1=a_b, op=OP.mult)
                        # reduce over n (outer free axis): view [B, D(step1), N(step D)]
                        gm_perm = AP(gm[:].tensor, gm[:].offset,
                                     [gm[:].ap[0], [1, D], [D, H1_CHUNK_N]])
                        red = hpool.tile([B, D], FP, tag="red")
                        nc.vector.tensor_reduce(out=red[:], in_=gm_perm,
                                                axis=mybir.AxisListType.X, op=OP.add)
                        nc.vector.tensor_tensor(
                            out=h1_acc[:, h * D:(h + 1) * D],
                            in0=h1_acc[:, h * D:(h + 1) * D], in1=red[:], op=OP.add)
                nc.sync.dma_start(out=h1_in[:], in_=h1_acc[:])
                nc.gpsimd.collective_compute(
                    "AllReduce", OP.add, replica_groups=rg,
                    ins=[h1_in[:]], outs=[h1_out[:]])

            # ---- tail (all graphs, redundant on every core) ----
            if stages >= 7:
                h1f = wpool.tile([B, W3_COLS], FP, tag="h1f")
                nc.sync.dma_start(out=h1f[:], in_=h1_out[:])
                h1r = wpool.tile([B, W3_COLS], FP, tag="h1r")
                nc.scalar.activation(out=h1r[:], in_=h1f[:], func=AF.Relu)
                h1s = cpool.tile([B, W3_COLS], FP, tag="h1s")
                nc.scalar.activation(out=h1s[:], in_=h1r[:], func=AF.Sqrt,
                                     bias=eps_sb[0:B, :])

                h2_acc = cpool.tile([B, W3_COLS], FP, tag="h2acc")
                nc.gpsimd.memset(h2_acc[:], 0.0)
                n_g2 = NH * G2_CHUNKS_PER_HEAD  # 24
                for dm in range(n_g2 // 6):     # 4 DMAs of 6 chunks
                    wchunk = hpool.tile([F, G2_CHUNK * 6], BF, tag="w2c")
                    c0 = dm * G2_CHUNK * 6
                    nc.sync.dma_start(out=wchunk[:], in_=Wg2[:, c0:c0 + G2_CHUNK * 6])
                    for s in range(6):
                        cidx = dm * 6 + s
                        h = cidx // G2_CHUNKS_PER_HEAD
                        ci = cidx % G2_CHUNKS_PER_HEAD
                        psg = ps_hyp.tile([B, G2_CHUNK], FP, space="PSUM", tag="psg")
                        nc.tensor.matmul(psg[:], lhsT=hgT_bf[:],
                                         rhs=wchunk[:, s * G2_CHUNK:(s + 1) * G2_CHUNK],
                                         start=True, stop=True)
                        ge = hpool.tile([B, G2_CHUNK], FP, tag="ge2")
                        nc.scalar.activation(out=ge[:], in_=psg[:], func=AF.Exp,
                                             bias=bg2_sb[0:B, :])
                        gsp = hpool.tile([B, G2_CHUNK], FP, tag="gsp2")
                        nc.scalar.activation(out=gsp[:], in_=ge[:], func=AF.Ln, bias=1.0)
                        d0 = ci * G2_CHUNK_D
                        hb = _bcast_free(h1s[:, h * D + d0:h * D + d0 + G2_CHUNK_D], D)
                        gm = hpool.tile([B, G2_CHUNK_D, D], FP, tag="gm2")
                        nc.vector.tensor_tensor(
                            out=gm[:], in0=gsp[:].rearrange("b (x e) -> b x e", e=D),
                            in1=hb, op=OP.mult)
                        gm_perm = AP(gm[:].tensor, gm[:].offset,
                                     [gm[:].ap[0], [1, D], [D, G2_CHUNK_D]])
                        red = hpool.tile([B, D], FP, tag="red2")
                        nc.vector.tensor_reduce(out=red[:], in_=gm_perm,
                                                axis=mybir.AxisListType.X, op=OP.add)
                        nc.vector.tensor_tensor(
                            out=h2_acc[:, h * D:(h + 1) * D],
                            in0=h2_acc[:, h * D:(h + 1) * D], in1=red[:], op=OP.add)
                # h2 = log1p(relu(h2_acc))
                h2r = wpool.tile([B, W3_COLS], FP, tag="h2r")
                nc.scalar.activation(out=h2r[:], in_=h2_acc[:], func=AF.Relu)
                h2l = wpool.tile([B, W3_COLS], FP, tag="h2l")
                nc.scalar.activation(out=h2l[:], in_=h2r[:], func=AF.Ln, bias=1.0)
                # w3 = softplus(hg @ Wg3 + bg3); q = sum_e h2*w3 per head; out = min_h
                psg3 = ps_hyp.tile([B, W3_COLS], FP, space="PSUM", tag="psg")
                nc.tensor.matmul(psg3[:], lhsT=hgT_bf[:], rhs=wg3_sb[:],
                                 start=True, stop=True)
                g3e = wpool.tile([B, W3_COLS], FP, tag="g3e")
                nc.scalar.activation(out=g3e[:], in_=psg3[:], func=AF.Exp,
                                     bias=bg3_sb[0:B, :])
                g3 = wpool.tile([B, W3_COLS], FP, tag="g3")
                nc.scalar.activation(out=g3[:], in_=g3e[:], func=AF.Ln, bias=1.0)
                qm = wpool.tile([B, W3_COLS], FP, tag="qm")
                nc.vector.tensor_tensor(out=qm[:], in0=h2l[:], in1=g3[:], op=OP.mult)
                qh = wpool.tile([B, NH], FP, tag="qh")
                nc.vector.tensor_reduce(out=qh[:],
                                        in_=qm[:].rearrange("b (h e) -> b h e", e=D),
                                        axis=mybir.AxisListType.X, op=OP.add)
                qmin = wpool.tile([B, 1], FP, tag="qmin")
                nc.vector.tensor_reduce(out=qmin[:], in_=qh[:],
                                        axis=mybir.AxisListType.X, op=OP.min)
                nc.sync.dma_start(out=out[:], in_=qmin[:])

    nc.finalize()
    return nc


def _prepare(inputs):
    """Host-side preprocessing: norms, edge sort/bucket/pad, per-core arrays."""
    x = np.asarray(inputs["x"], np.float32)
    edge_index = np.asarray(inputs["edge_index"])
    batch_index = np.asarray(inputs["batch_index"])
    actions = np.asarray(inputs["actions"], np.float32)
    assert np.array_equal(batch_index.astype(np.int64),
                          np.repeat(np.arange(B, dtype=np.int64), N_PER)), \
        "kernel assumes contiguous equal-size graphs"

    src = np.concatenate([edge_index[0], np.arange(TOTAL, dtype=edge_index.dtype)])
    dst = np.concatenate([edge_index[1], np.arange(TOTAL, dtype=edge_index.dtype)])
    deg = np.bincount(dst, minlength=TOTAL).astype(np.float32)
    dinv = np.where(deg > 0, 1.0 / np.sqrt(deg), 0.0).astype(np.float32)
    norm = dinv[src] * dinv[dst]

    order = np.argsort(dst, kind="stable")
    src_s = src[order].astype(np.int32)
    dst_s = dst[order].astype(np.int32)
    norm_s = norm[order]

    n_groups = NCORES * TILES_PER_CORE
    # group of each edge = dst // TILE_N  (125-node tiles, globally numbered)
    grp = dst_s // TILE_N
    counts = np.bincount(grp, minlength=n_groups)
    G = int(np.ceil(counts.max() / 128) * 128)
    starts = np.zeros(n_groups + 1, np.int64)
    np.cumsum(counts, out=starts[1:])

    per_core = []
    for k in range(NCORES):
        src_p = np.zeros((TILES_PER_CORE, G), np.int16)
        dstl_p = np.zeros((TILES_PER_CORE, G), np.float32)
        norm_p = np.zeros((TILES_PER_CORE, G), np.float32)
        for t in range(TILES_PER_CORE):
            gidx = k * TILES_PER_CORE + t
            s, e = starts[gidx], starts[gidx + 1]
            n = e - s
            src_p[t, :n] = src_s[s:e]
            dstl_p[t, :n] = (dst_s[s:e] - gidx * TILE_N).astype(np.float32)
            norm_p[t, :n] = norm_s[s:e]
        flat_src = src_p.reshape(-1)
        # dma_gather idx layout: [128, cols], row 16c+p col s -> idx[s*16+p], 8 Q7 replicas
        idxw = np.tile(flat_src.reshape(-1, 16).T, (8, 1)).copy()
        # dstl/norm layout: [128, cols], row p col c -> edge c*128+p
        dstl = dstl_p.reshape(-1, 128).T.astype(BF16).copy()
        normv = norm_p.reshape(-1, 128).T.astype(BF16).copy()
        per_core.append((idxw, dstl, normv))

    iota = np.tile(np.arange(TILE_N, dtype=np.float32), (128, 1)).astype(BF16)

    # pool weights: c[v, g] = sum of norm over edges v -> graph g (incl loops)
    gof = src.astype(np.int64) * B + dst.astype(np.int64) // N_PER
    c_full = np.bincount(gof, weights=norm.astype(np.float64),
                         minlength=TOTAL * B).reshape(TOTAL, B).astype(np.float32)

    bg1 = np.asarray(inputs["bg1"], np.float32)
    bg2 = np.asarray(inputs["bg2"], np.float32)
    bg3 = np.asarray(inputs["bg3"], np.float32)
    assert np.all(bg1 == bg1.flat[0]) and np.all(bg2 == bg2.flat[0]) \
        and np.all(bg3 == bg3.flat[0]), "generator biases must be constant"

    x_tbl = np.ascontiguousarray(x.astype(BF16))
    w1_full = np.asarray(inputs["Wg1"], np.float32).reshape(F, NH, N_PER, D)
    Wc1 = np.asarray(inputs["Wc1"], np.float32).astype(BF16)
    Wc2 = np.asarray(inputs["Wc2"], np.float32).astype(BF16)
    Wc3 = np.asarray(inputs["Wc3"], np.float32).astype(BF16)
    Wg2b = np.asarray(inputs["Wg2"], np.float32).astype(BF16)
    Wg3b = np.asarray(inputs["Wg3"], np.float32).astype(BF16)

    in_maps = []
    for k in range(NCORES):
        idxw, dstl, normv = per_core[k]
        w1s = np.ascontiguousarray(
            w1_full[:, :, k * N_SLICE:(k + 1) * N_SLICE, :]
            .reshape(F, W1_COLS)).astype(BF16)
        blk = c_full[k * NODES_PER_CORE:(k + 1) * NODES_PER_CORE]
        tmp = np.zeros((TILES_PER_CORE, 128, B), np.float32)
        tmp[:, :TILE_N, :] = blk.reshape(TILES_PER_CORE, TILE_N, B)
        cown = (tmp.transpose(1, 0, 2).reshape(128, TILES_PER_CORE * B)
                .astype(BF16).copy())
        in_maps.append(dict(
            x_tbl=x_tbl,
            idxw=idxw, dstl=dstl, normv=normv,
            Wc1=Wc1, Wc2=Wc2, Wc3=Wc3,
            bc1=np.asarray(inputs["bc1"], np.float32).reshape(F, 1),
            bc2=np.asarray(inputs["bc2"], np.float32).reshape(F, 1),
            bc3=np.asarray(inputs["bc3"], np.float32).reshape(F, 1),
            cown=cown,
            bc2r=np.asarray(inputs["bc2"], np.float32).reshape(1, F).astype(BF16),
            w1s=w1s,
            Wg2=Wg2b,
            Wg3=Wg3b,
            acts=np.ascontiguousarray(actions[:, k * N_SLICE:(k + 1) * N_SLICE]),
            iota=iota,
        ))
    return G, float(bg1.flat[0]), float(bg2.flat[0]), float(bg3.flat[0]), in_maps


class _Runner:
    """Compiled SPMD executable reusable across calls (jit cache keyed here)."""

    def __init__(self, nc):
        import jax
        from concourse import bass2jax
        from jax.experimental.shard_map import shard_map
        from jax.sharding import Mesh, PartitionSpec

        bass2jax.install_neuronx_cc_hook()
        self.jax = jax
        part_name = nc.partition_id_tensor.name if nc.partition_id_tensor else None
        in_names, out_names, out_avals, zero_outs = [], [], [], []
        for alloc in nc.m.functions[0].allocations:
            if not isinstance(alloc, mybir.MemoryLocationSet):
                continue
            name = alloc.memorylocations[0].name
            if alloc.kind == "ExternalInput":
                if name != part_name:
                    in_names.append(name)
            elif alloc.kind == "ExternalOutput":
                out_names.append(name)
                shape = tuple(alloc.tensor_shape)
                dtype = mybir.dt.np(alloc.dtype)
                out_avals.append(jax.core.ShapedArray(shape, dtype))
                zero_outs.append(np.zeros(shape, dtype))
        self.in_names, self.out_names = in_names, out_names
        self.zero_outs = zero_outs
        n_params, n_outs = len(in_names), len(out_names)

        bind_names = in_names + out_names + ([part_name] if part_name else [])

        def _body(*args):
            operands = list(args)
            if part_name:
                operands.append(bass2jax.partition_id_tensor())
            outs = bass2jax._bass_exec_p.bind(
                *operands,
                out_avals=tuple(out_avals),
                in_names=tuple(bind_names),
                out_names=tuple(out_names),
                lowering_input_output_aliases=(),
                sim_require_finite=False,
                sim_require_nnan=False,
                nc=nc,
            )
            return tuple(outs)

        devices = jax.devices()[:NCORES]
        mesh = Mesh(np.asarray(devices), ("core",))
        self.mesh = mesh
        self.PartitionSpec = PartitionSpec
        self.fn = jax.jit(
            shard_map(_body, mesh=mesh,
                      in_specs=(PartitionSpec("core"),) * (n_params + n_outs),
                      out_specs=(PartitionSpec("core"),) * n_outs,
                      check_rep=False),
            donate_argnums=tuple(range(n_params, n_params + n_outs)),
            keep_unused=True)

    def concat_inputs(self, in_maps):
        return [np.concatenate([np.asarray(m[n]) for m in in_maps], axis=0)
                for n in self.in_names]

    def run(self, concat_in):
        zeros = [np.zeros((NCORES * z.shape[0], *z.shape[1:]), z.dtype)
                 for z in self.zero_outs]
        out_arrs = self.fn(*concat_in, *zeros)
        return [np.asarray(a) for a in out_arrs]


def _get_runner(G, bg1v, bg2v, bg3v):
    key = (G, bg1v, bg2v, bg3v)
    if key not in _cache:
        nc = build_program(G, bg1v, bg2v, bg3v)
        _cache[key] = _Runner(nc)
    return _cache[key]


def kernel(**inputs):
    G, bg1v, bg2v, bg3v, in_maps = _prepare(inputs)
    runner = _get_runner(G, bg1v, bg2v, bg3v)
    outs = runner.run(runner.concat_inputs(in_maps))
    # out tensor is [NCORES*B, 1]; every core computed the full [B] result
    return outs[0].reshape(NCORES, B)[0].astype(np.float32)


# revision 14
# speedup vs baseline: 7.3561x; 1.0136x over previous
"""Trainium2 Bass kernel for CDSQN (3-layer GCN + hypernetwork Q-head).

Contract: kernel(**inputs) takes the FULL unsharded inputs (numpy) and
returns the FULL [B] float32 output. Internally shards across 8
NeuronCores: nodes/edges by dst range (4000 nodes = 4 graphs per core),
GCN weights replicated, hypernetwork Wg1 sharded over the node axis.

Structure (v2):
  layer 1: gather raw-x messages (bf16) -> norm-weighted aggregate via
           selection-matrix matmuls -> Wc1 post-multiply (GCN linearity:
           A(xW) = (Ax)W) -> relu -> t2 = h1@Wc2 -> AllGather tbl2
  layer 2: gather tbl2 messages -> aggregate -> relu -> AllGather h2 tbl3
  layer 3 + mean-pool are fused into one linear op: pool_g depends
           linearly on h2, so pool = Wc3^T (tbl3^T c) / N + bc3 where
           c[v,g] = sum of edge norms from v into graph g (host-built).
           This removes the third gather pass and the h_g AllGather.
  hypernet: w1 n-slice partial h1 per core -> AllReduce -> tail.
All message tables / weight matmuls run in bf16 (fp32 PSUM accumulate).
"""
import sys

sys.path.insert(0, "/opt/trn_rl_repo")

import ml_dtypes
import numpy as np

import concourse.bacc as bacc
import concourse.bass as bass
import concourse.mybir as mybir
import concourse.tile as tile
from concourse.ap import AP
from concourse.masks import make_identity

# ---- problem constants (hardcoded per spec) ----
B = 32
N_PER = 1000
TOTAL = B * N_PER          # 32000
E = 512000
F = 128                    # node_feat_dim == hidden_dim
D = 64
NH = 3
EPS = 1e-6

NCORES = 8
NODES_PER_CORE = TOTAL // NCORES       # 4000
GRAPHS_PER_CORE = B // NCORES          # 4
TILE_N = 125                           # dst nodes per output tile
TILES_PER_CORE = NODES_PER_CORE // TILE_N   # 32
N_SLICE = N_PER // NCORES              # 125 (hypernet n-shard per core)
W1_COLS = NH * N_SLICE * D             # 24000
W2_COLS = NH * D * D                   # 12288
W3_COLS = NH * D                       # 192
H1_CHUNK_N = 5                         # n values per hypernet chunk
H1_CHUNK = H1_CHUNK_N * D              # 320 cols, <=512 psum limit
H1_CHUNKS_PER_HEAD = N_SLICE // H1_CHUNK_N   # 25
H1_DMA_GROUP = 5                       # matmul-chunks per DMA
G2_CHUNK_D = 8
G2_CHUNK = G2_CHUNK_D * D              # 512
G2_CHUNKS_PER_HEAD = D // G2_CHUNK_D   # 8
PTILES = TOTAL // 128                  # 250 pool tiles
PT_CHUNK = 10                          # pool tiles per DMA
BF16 = ml_dtypes.bfloat16

FP = mybir.dt.float32
BF = mybir.dt.bfloat16
AF = mybir.ActivationFunctionType
OP = mybir.AluOpType

_cache = {}


def _bcast_free(ap, n):
    """Append a broadcast (step 0) innermost free dim of size n."""
    return AP(ap.tensor, ap.offset, list(ap.ap) + [[0, n]])


def build_program(G, bg1v, bg2v, bg3v, stages=99, variant=""):
    """Build the SPMD program (one NEFF, runs on all 8 cores)."""
    GCOLS16 = G // 16                   # idx cols per group
    GT = G // 128                       # edge tiles (matmuls) per group
    GCOLS = TILES_PER_CORE * GT         # dstl/norm cols per core

    nc = bacc.Bacc("TRN2", target_bir_lowering=False, debug=False,
                   enable_asserts=False, num_devices=NCORES)

    # ---- per-core inputs ----
    x_tbl = nc.dram_tensor("x_tbl", [TOTAL, F], BF, kind="ExternalInput")
    idxw = nc.dram_tensor("idxw", [128, TILES_PER_CORE * GCOLS16], mybir.dt.int16,
                          kind="ExternalInput")
    dstl = nc.dram_tensor("dstl", [128, GCOLS], BF, kind="ExternalInput")
    normv = nc.dram_tensor("normv", [128, GCOLS], BF, kind="ExternalInput")
    Wc = [nc.dram_tensor(f"Wc{i}", [F, F], BF, kind="ExternalInput") for i in (2, 3)]
    bc = [nc.dram_tensor(f"bc{i}", [F, 1], FP, kind="ExternalInput") for i in (1, 2, 3)]
    cown = nc.dram_tensor("cown", [128, TILES_PER_CORE * B], BF,
                          kind="ExternalInput")
    bc2r = nc.dram_tensor("bc2r", [1, F], BF, kind="ExternalInput")
    w1s = nc.dram_tensor("w1s", [F, W1_COLS], BF, kind="ExternalInput")
    Wg2 = nc.dram_tensor("Wg2", [F, W2_COLS], BF, kind="ExternalInput")
    Wg3 = nc.dram_tensor("Wg3", [F, W3_COLS], BF, kind="ExternalInput")
    acts = nc.dram_tensor("acts", [B, N_SLICE], FP, kind="ExternalInput")
    iota = nc.dram_tensor("iota", [128, TILE_N], BF, kind="ExternalInput")
    out = nc.dram_tensor("out", [B, 1], FP, kind="ExternalOutput")

    rg = [list(range(NCORES))]

    with tile.TileContext(nc) as tc:
        with tc.tile_pool(name="const", bufs=1) as cpool, \
             tc.tile_pool(name="meta", bufs=1) as mpool, \
             tc.tile_pool(name="msgs",
                          bufs=(8 if "b8" in variant else
                                6 if "b6" in variant else 4)) as gpool, \
             tc.tile_pool(name="work", bufs=4) as wpool, \
             tc.tile_pool(name="hyp", bufs=3) as hpool, \
             tc.tile_pool(name="ps_agg", bufs=2, space="PSUM") as ps_agg, \
             tc.tile_pool(name="ps_feat", bufs=2, space="PSUM") as ps_feat, \
             tc.tile_pool(name="ps_hyp", bufs=2, space="PSUM") as ps_hyp, \
             tc.tile_pool(name="dram", bufs=1, space="DRAM") as dr:

            # ---- DRAM internal tensors (collective bounce buffers) ----
            ag2_in = dr.tile([NODES_PER_CORE, F], BF, tag="ag2_in", name="ag2_in")
            tbl2 = dr.tile([TOTAL, F], BF, addr_space="Shared", tag="tbl2",
                           name="tbl2")
            tbl2_loc = (dr.tile([TOTAL, F], BF, tag="tbl2loc", name="tbl2_loc")
                        if "tloc" in variant else None)
            y_in = dr.tile([F, B], FP, tag="y_in")
            y_out = dr.tile([F, B], FP, addr_space="Shared", tag="y_out")
            h1_in = dr.tile([B, W3_COLS], FP, tag="h1_in")
            h1_out = dr.tile([B, W3_COLS], FP, addr_space="Shared", tag="h1_out")

            # ---- load constants ----
            Wc_sb = []
            bc_sb = []
            for i in range(2):
                w = cpool.tile([F, F], BF, tag=f"wc{i}")
                nc.sync.dma_start(out=w[:], in_=Wc[i][:])
                Wc_sb.append(w)
            for i in range(3):
                b_ = cpool.tile([F, 1], FP, tag=f"bc{i}")
                nc.sync.dma_start(out=b_[:], in_=bc[i][:])
                bc_sb.append(b_)
            iota_sb = cpool.tile([128, TILE_N], BF, tag="iota")
            nc.sync.dma_start(out=iota_sb[:], in_=iota[:])
            bg1_sb = cpool.tile([128, 1], FP, tag="bg1c")
            nc.gpsimd.memset(bg1_sb[:], bg1v)
            bg2_sb = cpool.tile([128, 1], FP, tag="bg2c")
            nc.gpsimd.memset(bg2_sb[:], bg2v)
            bg3_sb = cpool.tile([128, 1], FP, tag="bg3c")
            nc.gpsimd.memset(bg3_sb[:], bg3v)
            eps_sb = cpool.tile([128, 1], FP, tag="epsc")
            nc.gpsimd.memset(eps_sb[:], EPS)
            ones_sb = cpool.tile([1, TILE_N], BF, tag="ones1")
            nc.gpsimd.memset(ones_sb[:], 1.0)
            bc2r_sb = cpool.tile([1, F], BF, tag="bc2r")
            nc.sync.dma_start(out=bc2r_sb[:], in_=bc2r[:])
            idxw_sb = mpool.tile([128, TILES_PER_CORE * GCOLS16], mybir.dt.int16)
            nc.sync.dma_start(out=idxw_sb[:], in_=idxw[:])
            dstl_sb = mpool.tile([128, GCOLS], BF)
            nc.sync.dma_start(out=dstl_sb[:], in_=dstl[:])
            normv_sb = mpool.tile([128, GCOLS], BF)
            nc.sync.dma_start(out=normv_sb[:], in_=normv[:])
            cown_sb = mpool.tile([128, TILES_PER_CORE * B], BF)
            nc.sync.dma_start(out=cown_sb[:], in_=cown[:])
            acts_sb = cpool.tile([B, N_SLICE], FP, tag="acts")
            nc.sync.dma_start(out=acts_sb[:], in_=acts[:])
            wg3_sb = cpool.tile([F, W3_COLS], BF, tag="wg3")
            nc.sync.dma_start(out=wg3_sb[:], in_=Wg3[:])

            def gcn_group(src_tbl, g, node_major=False):
                """Gather + norm-weighted aggregate for dst tile g.
                Feature-major: PSUM [F, TILE_N]. Node-major: PSUM
                [TILE_N, F] with bc2 bias folded in via a rank-1 matmul."""
                msgs = gpool.tile([128, GT, F], BF, tag="msgs")
                nc.gpsimd.dma_gather(
                    out_ap=msgs[:], in_ap=src_tbl[:],
                    idxs_ap=idxw_sb[:, g * GCOLS16:(g + 1) * GCOLS16],
                    num_idxs=G, num_idxs_reg=G, elem_size=F,
                    single_packet=("sp1" in variant))
                st = wpool.tile([128, GT, TILE_N], BF, tag="st",
                                bufs=(4 if "b8" in variant else
                                      3 if "b6" in variant else 2))
                iota_bc = AP(iota_sb[:].tensor, iota_sb[:].offset,
                             [iota_sb[:].ap[0], [0, GT], [1, TILE_N]])
                nc.vector.tensor_tensor(
                    out=st[:], in0=iota_bc,
                    in1=_bcast_free(dstl_sb[:, g * GT:(g + 1) * GT], TILE_N),
                    op=OP.is_equal)
                nc.vector.tensor_tensor(
                    out=st[:], in0=st[:],
                    in1=_bcast_free(normv_sb[:, g * GT:(g + 1) * GT], TILE_N),
                    op=OP.mult)
                if node_major:
                    agg = ps_agg.tile([TILE_N, F], FP, space="PSUM", tag="agg")
                    for j in range(GT):
                        nc.tensor.matmul(agg[:], lhsT=st[:, j, :], rhs=msgs[:, j, :],
                                         start=(j == 0), stop=False)
                    nc.tensor.matmul(agg[:], lhsT=ones_sb[:], rhs=bc2r_sb[:],
                                     start=False, stop=True)
                else:
                    agg = ps_agg.tile([F, TILE_N], FP, space="PSUM", tag="agg")
                    for j in range(GT):
                        nc.tensor.matmul(agg[:], lhsT=msgs[:, j, :], rhs=st[:, j, :],
                                         start=(j == 0), stop=(j == GT - 1))
                return agg

            # ---- layer 1: aggregate raw x, h1 = relu(Wc1^T agg + b1),
            #      t2 = h1 @ Wc2 (own nodes) ----
            if stages >= 1:
                for g in range(TILES_PER_CORE):
                    agg = gcn_group(x_tbl, g)
                    hT = wpool.tile([F, TILE_N], BF, tag="hT")
                    nc.scalar.activation(out=hT[:], in_=agg[:], func=AF.Relu,
                                         bias=bc_sb[0][:])
                    ps_t2 = ps_feat.tile([TILE_N, F], FP, space="PSUM", tag="feat")
                    nc.tensor.matmul(ps_t2[:], lhsT=hT[:], rhs=Wc_sb[0][:],
                                     start=True, stop=True)
                    t2sb = wpool.tile([TILE_N, F], BF, tag="t2sb")
                    nc.vector.tensor_copy(out=t2sb[:], in_=ps_t2[:])
                    nc.sync.dma_start(
                        out=ag2_in[g * TILE_N:(g + 1) * TILE_N, :], in_=t2sb[:])
            if stages >= 2:
                nc.gpsimd.collective_compute(
                    "AllGather", OP.bypass, replica_groups=rg,
                    ins=[ag2_in[:]], outs=[tbl2[:]])
                if "tloc" in variant:
                    nc.sync.dma_start(out=tbl2_loc[:], in_=tbl2[:])

            # ---- layer 2: node-major agg, h2 = relu(agg + b2), local pool
            #      partial y += h2^T c_own accumulated in PSUM ----
            if stages >= 3:
                y_ps = ps_agg.tile([F, B], FP, space="PSUM", tag="ypool", bufs=1)
                h2all = mpool.tile([128, TILES_PER_CORE, F], BF)
                for g in range(TILES_PER_CORE):
                    agg = gcn_group(
                        tbl2_loc if "tloc" in variant else tbl2,
                        g, node_major=True)
                    nc.scalar.activation(out=h2all[0:TILE_N, g, :], in_=agg[:],
                                         func=AF.Relu)
                for g in range(TILES_PER_CORE):
                    nc.tensor.matmul(y_ps[:], lhsT=h2all[0:TILE_N, g, :],
                                     rhs=cown_sb[0:TILE_N, g * B:(g + 1) * B],
                                     start=(g == 0), stop=(g == TILES_PER_CORE - 1))
            if stages >= 4:
                y_sb = wpool.tile([F, B], FP, tag="ysb")
                nc.vector.tensor_copy(out=y_sb[:], in_=y_ps[:])
                nc.sync.dma_start(out=y_in[:], in_=y_sb[:])
                nc.gpsimd.collective_compute(
                    "AllReduce", OP.add, replica_groups=rg,
                    ins=[y_in[:]], outs=[y_out[:]])

            # ---- layer 3 + mean pool tail: hgT = Wc3^T y / N_PER + bc3 ----
            if stages >= 5:
                y_f = wpool.tile([F, B], FP, tag="yf")
                nc.sync.dma_start(out=y_f[:], in_=y_out[:])
                y_bf = wpool.tile([F, B], BF, tag="ybf")
                nc.vector.tensor_copy(out=y_bf[:], in_=y_f[:])
                hg_ps = ps_feat.tile([F, B], FP, space="PSUM", tag="feat")
                nc.tensor.matmul(hg_ps[:], lhsT=Wc_sb[1][:], rhs=y_bf[:],
                                 start=True, stop=True)
                hgT_bf = cpool.tile([F, B], BF, tag="hgTb")
                nc.scalar.activation(out=hgT_bf[:], in_=hg_ps[:], func=AF.Identity,
                                     bias=bc_sb[2][:], scale=1.0 / N_PER)

            if stages < 8:
                out_t = wpool.tile([B, 1], FP, tag="qmin")
                nc.gpsimd.memset(out_t[:], 0.0)
                nc.sync.dma_start(out=out[:], in_=out_t[:])

            # ---- hypernet h1 partial (own n-slice of w1), all-reduce ----
            if stages >= 6:
                h1_acc = cpool.tile([B, W3_COLS], FP, tag="h1acc")
                nc.gpsimd.memset(h1_acc[:], 0.0)
                n_dma = H1_CHUNKS_PER_HEAD * NH // H1_DMA_GROUP  # 15
                for dm in range(n_dma):
                    wchunk = hpool.tile([F, H1_CHUNK * H1_DMA_GROUP], BF, tag="w1c")
                    c0 = dm * H1_CHUNK * H1_DMA_GROUP
                    nc.sync.dma_start(out=wchunk[:],
                                      in_=w1s[:, c0:c0 + H1_CHUNK * H1_DMA_GROUP])
                    for s in range(H1_DMA_GROUP):
                        cidx = dm * H1_DMA_GROUP + s          # global chunk index
                        h = cidx // H1_CHUNKS_PER_HEAD        # head
                        ci = cidx % H1_CHUNKS_PER_HEAD        # chunk within head
                        psg = ps_hyp.tile([B, H1_CHUNK], FP, space="PSUM", tag="psg")
                        nc.tensor.matmul(psg[:], lhsT=hgT_bf[:],
                                         rhs=wchunk[:, s * H1_CHUNK:(s + 1) * H1_CHUNK],
                                         start=True, stop=True)
                        ge = hpool.tile([B, H1_CHUNK], FP, tag="ge")
                        nc.scalar.activation(out=ge[:], in_=psg[:], func=AF.Exp,
                                             bias=bg1_sb[0:B, :])
                        gsp = hpool.tile([B, H1_CHUNK], FP, tag="gsp")
                        nc.scalar.activation(out=gsp[:], in_=ge[:], func=AF.Ln, bias=1.0)
                        # multiply by actions (broadcast over d), reduce over n
                        n0 = ci * H1_CHUNK_N
                        a_b = _bcast_free(acts_sb[:, n0:n0 + H1_CHUNK_N], D)
                        gm = hpool.tile([B, H1_CHUNK_N, D], FP, tag="gm")
                        nc.vector.tensor_tensor(
                            out=gm[:], in0=gsp[:].rearrange("b (n d) -> b n d", d=D),
                            in1=a_b, op=OP.mult)
                        # reduce over n (outer free axis): view [B, D(step1), N(step D)]
                        gm_perm = AP(gm[:].tensor, gm[:].offset,
                                     [gm[:].ap[0], [1, D], [D, H1_CHUNK_N]])
                        red = hpool.tile([B, D], FP, tag="red")
                        nc.vector.tensor_reduce(out=red[:], in_=gm_perm,
                                                axis=mybir.AxisListType.X, op=OP.add)
                        nc.vector.tensor_tensor(
                            out=h1_acc[:, h * D:(h + 1) * D],
                            in0=h1_acc[:, h * D:(h + 1) * D], in1=red[:], op=OP.add)
                nc.sync.dma_start(out=h1_in[:], in_=h1_acc[:])
                nc.gpsimd.collective_compute(
                    "AllReduce", OP.add, replica_groups=rg,
                    ins=[h1_in[:]], outs=[h1_out[:]])

            # ---- tail (all graphs, redundant on every core) ----
            if stages >= 7:
                h1f = wpool.tile([B, W3_COLS], FP, tag="h1f")
                nc.sync.dma_start(out=h1f[:], in_=h1_out[:])
                h1r = wpool.tile([B, W3_COLS], FP, tag="h1r")
                nc.scalar.activation(out=h1r[:], in_=h1f[:], func=AF.Relu)
                h1s = cpool.tile([B, W3_COLS], FP, tag="h1s")
                nc.scalar.activation(out=h1s[:], in_=h1r[:], func=AF.Sqrt,
                                     bias=eps_sb[0:B, :])

                h2_acc = cpool.tile([B, W3_COLS], FP, tag="h2acc")
                nc.gpsimd.memset(h2_acc[:], 0.0)
                n_g2 = NH * G2_CHUNKS_PER_HEAD  # 24
                for dm in range(n_g2 // 6):     # 4 DMAs of 6 chunks
                    wchunk = hpool.tile([F, G2_CHUNK * 6], BF, tag="w2c")
                    c0 = dm * G2_CHUNK * 6
                    nc.sync.dma_start(out=wchunk[:], in_=Wg2[:, c0:c0 + G2_CHUNK * 6])
                    for s in range(6):
                        cidx = dm * 6 + s
                        h = cidx // G2_CHUNKS_PER_HEAD
                        ci = cidx % G2_CHUNKS_PER_HEAD
                        psg = ps_hyp.tile([B, G2_CHUNK], FP, space="PSUM", tag="psg")
                        nc.tensor.matmul(psg[:], lhsT=hgT_bf[:],
                                         rhs=wchunk[:, s * G2_CHUNK:(s + 1) * G2_CHUNK],
                                         start=True, stop=True)
                        ge = hpool.tile([B, G2_CHUNK], FP, tag="ge2")
                        nc.scalar.activation(out=ge[:], in_=psg[:], func=AF.Exp,
                                             bias=bg2_sb[0:B, :])
                        gsp = hpool.tile([B, G2_CHUNK], FP, tag="gsp2")
                        nc.scalar.activation(out=gsp[:], in_=ge[:], func=AF.Ln, bias=1.0)
                        d0 = ci * G2_CHUNK_D
                        hb = _bcast_free(h1s[:, h * D + d0:h * D + d0 + G2_CHUNK_D], D)
                        gm = hpool.tile([B, G2_CHUNK_D, D], FP, tag="gm2")
                        nc.vector.tensor_tensor(
                            out=gm[:], in0=gsp[:].rearrange("b (x e) -> b x e", e=D),
                            in1=hb, op=OP.mult)
                        gm_perm = AP(gm[:].tensor, gm[:].offset,
                                     [gm[:].ap[0], [1, D], [D, G2_CHUNK_D]])
                        red = hpool.tile([B, D], FP, tag="red2")
                        nc.vector.tensor_reduce(out=red[:], in_=gm_perm,
                                                axis=mybir.AxisListType.X, op=OP.add)
                        nc.vector.tensor_tensor(
                            out=h2_acc[:, h * D:(h + 1) * D],
                            in0=h2_acc[:, h * D:(h + 1) * D], in1=red[:], op=OP.add)
                # h2 = log1p(relu(h2_acc))
                h2r = wpool.tile([B, W3_COLS], FP, tag="h2r")
                nc.scalar.activation(out=h2r[:], in_=h2_acc[:], func=AF.Relu)
                h2l = wpool.tile([B, W3_COLS], FP, tag="h2l")
                nc.scalar.activation(out=h2l[:], in_=h2r[:], func=AF.Ln, bias=1.0)
                # w3 = softplus(hg @ Wg3 + bg3); q = sum_e h2*w3 per head; out = min_h
                psg3 = ps_hyp.tile([B, W3_COLS], FP, space="PSUM", tag="psg")
                nc.tensor.matmul(psg3[:], lhsT=hgT_bf[:], rhs=wg3_sb[:],
                                 start=True, stop=True)
                g3e = wpool.tile([B, W3_COLS], FP, tag="g3e")
                nc.scalar.activation(out=g3e[:], in_=psg3[:], func=AF.Exp,
                                     bias=bg3_sb[0:B, :])
                g3 = wpool.tile([B, W3_COLS], FP, tag="g3")
                nc.scalar.activation(out=g3[:], in_=g3e[:], func=AF.Ln, bias=1.0)
                qm = wpool.tile([B, W3_COLS], FP, tag="qm")
                nc.vector.tensor_tensor(out=qm[:], in0=h2l[:], in1=g3[:], op=OP.mult)
                qh = wpool.tile([B, NH], FP, tag="qh")
                nc.vector.tensor_reduce(out=qh[:],
                                        in_=qm[:].rearrange("b (h e) -> b h e", e=D),
                                        axis=mybir.AxisListType.X, op=OP.add)
                qmin = wpool.tile([B, 1], FP, tag="qmin")
                nc.vector.tensor_reduce(out=qmin[:], in_=qh[:],
                                        axis=mybir.AxisListType.X, op=OP.min)
                nc.sync.dma_start(out=out[:], in_=qmin[:])

    nc.finalize()
    return nc


def _prepare(inputs):
    """Host-side preprocessing: norms, edge sort/bucket/pad, per-core arrays."""
    x = np.asarray(inputs["x"], np.float32)
    edge_index = np.asarray(inputs["edge_index"])
    batch_index = np.asarray(inputs["batch_index"])
    actions = np.asarray(inputs["actions"], np.float32)
    assert np.array_equal(batch_index.astype(np.int64),
                          np.repeat(np.arange(B, dtype=np.int64), N_PER)), \
        "kernel assumes contiguous equal-size graphs"

    src = np.concatenate([edge_index[0], np.arange(TOTAL, dtype=edge_index.dtype)])
    dst = np.concatenate([edge_index[1], np.arange(TOTAL, dtype=edge_index.dtype)])
    deg = np.bincount(dst, minlength=TOTAL).astype(np.float32)
    dinv = np.where(deg > 0, 1.0 / np.sqrt(deg), 0.0).astype(np.float32)
    norm = dinv[src] * dinv[dst]

    order = np.argsort(dst, kind="stable")
    src_s = src[order].astype(np.int32)
    dst_s = dst[order].astype(np.int32)
    norm_s = norm[order]

    n_groups = NCORES * TILES_PER_CORE
    # group of each edge = dst // TILE_N  (125-node tiles, globally numbered)
    grp = dst_s // TILE_N
    counts = np.bincount(grp, minlength=n_groups)
    G = int(np.ceil(counts.max() / 128) * 128)
    starts = np.zeros(n_groups + 1, np.int64)
    np.cumsum(counts, out=starts[1:])

    per_core = []
    for k in range(NCORES):
        src_p = np.zeros((TILES_PER_CORE, G), np.int16)
        dstl_p = np.zeros((TILES_PER_CORE, G), np.float32)
        norm_p = np.zeros((TILES_PER_CORE, G), np.float32)
        for t in range(TILES_PER_CORE):
            gidx = k * TILES_PER_CORE + t
            s, e = starts[gidx], starts[gidx + 1]
            n = e - s
            src_p[t, :n] = src_s[s:e]
            dstl_p[t, :n] = (dst_s[s:e] - gidx * TILE_N).astype(np.float32)
            norm_p[t, :n] = norm_s[s:e]
        flat_src = src_p.reshape(-1)
        # dma_gather idx layout: [128, cols], row 16c+p col s -> idx[s*16+p], 8 Q7 replicas
        idxw = np.tile(flat_src.reshape(-1, 16).T, (8, 1)).copy()
        # dstl/norm layout: [128, cols], row p col c -> edge c*128+p
        dstl = dstl_p.reshape(-1, 128).T.astype(BF16).copy()
        normv = norm_p.reshape(-1, 128).T.astype(BF16).copy()
        per_core.append((idxw, dstl, normv))

    iota = np.tile(np.arange(TILE_N, dtype=np.float32), (128, 1)).astype(BF16)

    # pool weights: c[v, g] = sum of norm over edges v -> graph g (incl loops)
    gof = src.astype(np.int64) * B + dst.astype(np.int64) // N_PER
    c_full = np.bincount(gof, weights=norm.astype(np.float64),
                         minlength=TOTAL * B).reshape(TOTAL, B).astype(np.float32)

    bg1 = np.asarray(inputs["bg1"], np.float32)
    bg2 = np.asarray(inputs["bg2"], np.float32)
    bg3 = np.asarray(inputs["bg3"], np.float32)
    assert np.all(bg1 == bg1.flat[0]) and np.all(bg2 == bg2.flat[0]) \
        and np.all(bg3 == bg3.flat[0]), "generator biases must be constant"

    x_tbl = np.ascontiguousarray(
        (x @ np.asarray(inputs["Wc1"], np.float32)).astype(BF16))
    w1_full = np.asarray(inputs["Wg1"], np.float32).reshape(F, NH, N_PER, D)
    Wc2 = np.asarray(inputs["Wc2"], np.float32).astype(BF16)
    Wc3 = np.asarray(inputs["Wc3"], np.float32).astype(BF16)
    Wg2b = np.asarray(inputs["Wg2"], np.float32).astype(BF16)
    Wg3b = np.asarray(inputs["Wg3"], np.float32).astype(BF16)

    in_maps = []
    for k in range(NCORES):
        idxw, dstl, normv = per_core[k]
        w1s = np.ascontiguousarray(
            w1_full[:, :, k * N_SLICE:(k + 1) * N_SLICE, :]
            .reshape(F, W1_COLS)).astype(BF16)
        blk = c_full[k * NODES_PER_CORE:(k + 1) * NODES_PER_CORE]
        tmp = np.zeros((TILES_PER_CORE, 128, B), np.float32)
        tmp[:, :TILE_N, :] = blk.reshape(TILES_PER_CORE, TILE_N, B)
        cown = (tmp.transpose(1, 0, 2).reshape(128, TILES_PER_CORE * B)
                .astype(BF16).copy())
        in_maps.append(dict(
            x_tbl=x_tbl,
            idxw=idxw, dstl=dstl, normv=normv,
            Wc2=Wc2, Wc3=Wc3,
            bc1=np.asarray(inputs["bc1"], np.float32).reshape(F, 1),
            bc2=np.asarray(inputs["bc2"], np.float32).reshape(F, 1),
            bc3=np.asarray(inputs["bc3"], np.float32).reshape(F, 1),
            cown=cown,
            bc2r=np.asarray(inputs["bc2"], np.float32).reshape(1, F).astype(BF16),
            w1s=w1s,
            Wg2=Wg2b,
            Wg3=Wg3b,
            acts=np.ascontiguousarray(actions[:, k * N_SLICE:(k + 1) * N_SLICE]),
            iota=iota,
        ))
    return G, float(bg1.flat[0]), float(bg2.flat[0]), float(bg3.flat[0]), in_maps


class _Runner:
    """Compiled SPMD executable reusable across calls (jit cache keyed here)."""

    def __init__(self, nc):
        import jax
        from concourse import bass2jax
        from jax.experimental.shard_map import shard_map
        from jax.sharding import Mesh, PartitionSpec

        bass2jax.install_neuronx_cc_hook()
        self.jax = jax
        part_name = nc.partition_id_tensor.name if nc.partition_id_tensor else None
        in_names, out_names, out_avals, zero_outs = [], [], [], []
        for alloc in nc.m.functions[0].allocations:
            if not isinstance(alloc, mybir.MemoryLocationSet):
                continue
            name = alloc.memorylocations[0].name
            if alloc.kind == "ExternalInput":
                if name != part_name:
                    in_names.append(name)
            elif alloc.kind == "ExternalOutput":
                out_names.append(name)
                shape = tuple(alloc.tensor_shape)
                dtype = mybir.dt.np(alloc.dtype)
                out_avals.append(jax.core.ShapedArray(shape, dtype))
                zero_outs.append(np.zeros(shape, dtype))
        self.in_names, self.out_names = in_names, out_names
        self.zero_outs = zero_outs
        n_params, n_outs = len(in_names), len(out_names)

        bind_names = in_names + out_names + ([part_name] if part_name else [])

        def _body(*args):
            operands = list(args)
            if part_name:
                operands.append(bass2jax.partition_id_tensor())
            outs = bass2jax._bass_exec_p.bind(
                *operands,
                out_avals=tuple(out_avals),
                in_names=tuple(bind_names),
                out_names=tuple(out_names),
                lowering_input_output_aliases=(),
                sim_require_finite=False,
                sim_require_nnan=False,
                nc=nc,
            )
            return tuple(outs)

        devices = jax.devices()[:NCORES]
        mesh = Mesh(np.asarray(devices), ("core",))
        self.mesh = mesh
        self.PartitionSpec = PartitionSpec
        self.fn = jax.jit(
            shard_map(_body, mesh=mesh,
                      in_specs=(PartitionSpec("core"),) * (n_params + n_outs),
                      out_specs=(PartitionSpec("core"),) * n_outs,
                      check_rep=False),
            donate_argnums=tuple(range(n_params, n_params + n_outs)),
            keep_unused=True)

    def concat_inputs(self, in_maps):
        return [np.concatenate([np.asarray(m[n]) for m in in_maps], axis=0)
                for n in self.in_names]

    def run(self, concat_in):
        zeros = [np.zeros((NCORES * z.shape[0], *z.shape[1:]), z.dtype)
                 for z in self.zero_outs]
        out_arrs = self.fn(*concat_in, *zeros)
        return [np.asarray(a) for a in out_arrs]


def _get_runner(G, bg1v, bg2v, bg3v):
    key = (G, bg1v, bg2v, bg3v)
    if key not in _cache:
        nc = build_program(G, bg1v, bg2v, bg3v)
        _cache[key] = _Runner(nc)
    return _cache[key]


def kernel(**inputs):
    G, bg1v, bg2v, bg3v, in_maps = _prepare(inputs)
    runner = _get_runner(G, bg1v, bg2v, bg3v)
    outs = runner.run(runner.concat_inputs(in_maps))
    # out tensor is [NCORES*B, 1]; every core computed the full [B] result
    return outs[0].reshape(NCORES, B)[0].astype(np.float32)
